# revision 1
# baseline (speedup 1.0000x reference)
"""LoRA MHA kernel for TRN2, batch x head-group parallel across 8 NeuronCores.

Problem: nn_LoRAMultiheadAttention (S=2048, B=2, E=1024, H=16, HD=64, rank=8).

Strategy
--------
* Host folds the LoRA low-rank update into the frozen weights:
  W_eff = W + (alpha/rank) * B @ A  (exact same math, rank-8 update).
* Hybrid sharding: core c handles batch c//4 and head group c%4 (4 heads,
  256 features).  Per-core input is x^T for its batch only (12 MB vs 24 MB
  for pure head-parallel); per-core output is a bf16 partial [1024, 2048]
  that the host sums within each 4-core batch group (+ out_b).
* Device dataflow:
    - Q^T, K^T = W_eff_c @ x^T + bias   ([feature, token], chunk-pipelined
      with the input DMA).
    - V computed directly token-major (lhsT = x^T tile, rhs = W_v^T slice),
      bias folded into the PSUM->SBUF move via a pre-replicated bias tile,
      laid out per head as [64 V cols | ones col] so the attn@V matmul also
      produces the softmax row-sum as output row 64 (no row-sum matmuls).
    - scores S^T = K_h @ Q_h^T per head into a 4-bank PSUM ring, two heads
      row-tiled via tile_position; exp(0.125 * S^T) on the Scalar engine in
      [128, 1024] groups (scores are O(+-3) so no max subtraction).
    - out^T = [V_h | 1] @ P_h^T accumulated over key tiles; row 64 = rowsum;
      reciprocal + PE broadcast + DVE multiply normalizes.
    - partial^T = W_out_c @ attn^T, DVE copy to bf16, DMA out per 2 f-tiles.
* Emission is software-pipelined by hand: the engines' sync is per-engine
  monotonic counters and their lookahead windows are shallow, so attn@V /
  normalize / out-proj are deferred closures drained into the next head
  pair's score+exp stream, and the K c1-3 / V / Q c1-3 projections are
  background items inside the first attention chunk.
"""

import sys
from collections import deque

import numpy as np

if "/opt/trn_rl_repo" not in sys.path:
    sys.path.insert(0, "/opt/trn_rl_repo")

import ml_dtypes  # noqa: E402

import concourse.bass as bass  # noqa: E402
from concourse import bacc  # noqa: E402
import concourse.mybir as mybir  # noqa: E402
import concourse.tile as tile  # noqa: E402
from concourse.bass_utils import run_bass_kernel_spmd  # noqa: E402

S, B, E = 2048, 2, 1024
H, HD = 16, 64
RANK = 8
LORA_SCALE = 16.0 / RANK
NCORES = 8
NHC = 4            # heads per core
FPC = NHC * HD     # features per core = 256
ET = E // 128      # 8 contraction tiles for the projections
QC = S // 512      # 4 query/token chunks of 512
KT = S // 128      # 16 key tiles of 128
TT = S // 128      # 16 token tiles of 128 (V layout)
OFT = E // 128     # 8 output-feature tiles

BF16 = mybir.dt.bfloat16
F32 = mybir.dt.float32
EXP = mybir.ActivationFunctionType.Exp

_PROGRAM = None


def _build_program():
    nc = bacc.Bacc(None)

    xq_d = nc.dram_tensor("xq", [128, QC, ET, 512], BF16, kind="ExternalInput")
    xk_d = nc.dram_tensor("xk", [128, QC, ET, 512], BF16, kind="ExternalInput")
    xv_d = nc.dram_tensor("xv", [128, ET, TT, 128], BF16, kind="ExternalInput")
    wq_d = nc.dram_tensor("wq", [128, ET, FPC], BF16, kind="ExternalInput")
    wk_d = nc.dram_tensor("wk", [128, ET, FPC], BF16, kind="ExternalInput")
    wv_d = nc.dram_tensor("wv", [128, ET, FPC], BF16, kind="ExternalInput")
    wo_d = nc.dram_tensor("wo", [128, 2, OFT, 128], BF16, kind="ExternalInput")
    bq_d = nc.dram_tensor("bq", [128, 2], F32, kind="ExternalInput")
    bk_d = nc.dram_tensor("bk", [128, 2], F32, kind="ExternalInput")
    bv_d = nc.dram_tensor("bv", [1, FPC], BF16, kind="ExternalInput")
    out_d = nc.dram_tensor("out", [128, OFT, QC, 512], BF16, kind="ExternalOutput")

    with tile.TileContext(nc) as tc:
        with tc.tile_pool(name="singles", bufs=1) as singles:
            # Q^T / K^T as per-chunk tiles: tile-exact dependencies, so the
            # first score groups don't inherit waits on later chunk writes
            # (the dep tracker merges column ranges within one tile)
            qt_c = [
                singles.tile([128, 2, 512], BF16, tag=f"qt{c}", name=f"qt{c}")
                for c in range(QC)
            ]
            kt_c = [
                singles.tile([128, 2, 512], BF16, tag=f"kt{c}", name=f"kt{c}")
                for c in range(QC)
            ]
            # token-major V: 16 token tiles x 4*(64 V | ones)
            v_sb = singles.tile([128, TT, 65 * NHC], BF16, tag="v")
            xv_sb = singles.tile([128, ET, TT, 128], BF16, tag="xv")
            wq_sb = singles.tile([128, ET, FPC], BF16, tag="wq")
            wk_sb = singles.tile([128, ET, FPC], BF16, tag="wk")
            wv_sb = singles.tile([128, ET, FPC], BF16, tag="wv")
            wo_sb = singles.tile([128, 2, OFT, 128], BF16, tag="wo")
            bq_sb = singles.tile([128, 2], F32, tag="bq")
            bk_sb = singles.tile([128, 2], F32, tag="bk")
            bv_sb = singles.tile([1, FPC], BF16, tag="bv")
            bvrep_sb = singles.tile([128, FPC], BF16, tag="bvrep")
            ones_sb = singles.tile([128, 128], BF16, tag="ones")

            # DMA queue order = dependency order: consumers wait on the DMA
            # *counter*, so anything queued before a tensor delays its users
            nc.sync.dma_start(out=wk_sb, in_=wk_d[:])
            nc.vector.memset(ones_sb, 1.0)

            # Single scope: 8 PSUM banks total.  rings (2x2 banks) double as
            # the K c0 / Q c0 projection accumulators; the po pool (2) is
            # attn@V only; the oprb pool (2) serves the PE warm-up, K c1-3 /
            # Q c1-3 / V projections, rb broadcast, and out-projection.
            with (
                tc.tile_pool(name="xchunks", bufs=6) as xpool,
                tc.tile_pool(name="ring_ps", bufs=2, space="PSUM") as ringp,
                tc.tile_pool(name="po_ps", bufs=2, space="PSUM") as pops,
                tc.tile_pool(name="oprb_ps", bufs=2, space="PSUM") as oprb,
                tc.tile_pool(name="p_sb", bufs=4) as ppool,
                tc.tile_pool(name="at_sb", bufs=2) as apool,
                tc.tile_pool(name="rec_sb", bufs=2) as rpool,
                tc.tile_pool(name="rbs_sb", bufs=3) as rbpool,
                tc.tile_pool(name="ob_sb", bufs=3) as opool,
            ):
                # one 2-bank score ring per head of a pair: independent
                # tiles so each head's WAR chain (exp -> next scores)
                # doesn't serialize against the other head's exp
                rings = [
                    ringp.tile([128, 2, 512], F32, tag="ring", name=f"ring{j}")
                    for j in range(2)
                ]

                def qk_mms(w_sb, xs, f, ps):
                    for e in range(ET):
                        nc.tensor.matmul(
                            ps,
                            w_sb[:, e, f * 128 : (f + 1) * 128],
                            xs[:, e, :],
                            start=(e == 0),
                            stop=(e == ET - 1),
                        )

                def proj_chunk_oprb_parts(x_d, w_sb, b_sb, dst, c, nm):
                    """Per-ftile closures for a projection chunk via oprb."""
                    xs = [None]

                    def part(f):
                        if f == 0:
                            xs[0] = xpool.tile(
                                [128, ET, 512], BF16, tag="x", name=nm
                            )
                            nc.sync.dma_start(
                                out=xs[0][:, 0:4, :], in_=x_d[:, c, 0:4, :]
                            )
                            nc.sync.dma_start(
                                out=xs[0][:, 4:8, :], in_=x_d[:, c, 4:8, :]
                            )
                        ps = oprb.tile([128, 512], F32, tag="oprb", name=f"{nm}_ps{f}")
                        qk_mms(w_sb, xs[0], f, ps)
                        nc.vector.tensor_scalar_add(
                            dst[:, f, :], ps, b_sb[:, f : f + 1]
                        )

                    return [lambda f=f: part(f) for f in range(2)]

                def v_dma(tg):
                    nc.sync.dma_start(
                        out=xv_sb[:, :, tg : tg + 4, :],
                        in_=xv_d[:, :, tg : tg + 4, :],
                    )

                vtt_emitted = [0]

                def v_tt(tt):
                    vtt_emitted[0] += 1
                    vp = oprb.tile([128, 512], F32, tag="oprb", name=f"vp{tt}")
                    for e in range(ET):
                        nc.tensor.matmul(
                            vp[:, 0:FPC],
                            xv_sb[:, e, tt, :],
                            wv_sb[:, e, :],
                            start=(e == 0),
                            stop=(e == ET - 1),
                        )
                    # bias folded into the PSUM->SBUF move (DVE add against
                    # the pre-replicated bias tile; same cost as a copy)
                    dst = v_sb.rearrange("p t (h c) -> p t h c", h=NHC)[:, tt, :, 0:64]
                    src = vp[:, 0:FPC].rearrange("p (h c) -> p h c", h=NHC)
                    brep = bvrep_sb.rearrange("p (h c) -> p h c", h=NHC)
                    nc.vector.tensor_add(dst, src, brep)

                # warm the PE p-state during the initial input-DMA wait
                warm = oprb.tile([128, 512], F32, tag="oprb", name="warm")
                for w in range(48):
                    nc.tensor.matmul(
                        warm[:, 0:128], ones_sb, ones_sb,
                        start=(w == 0), stop=(w == 47),
                    )
                # ones column (index 64) of every head group in every token
                # tile; emitted after the warm-up so the warm-up's DVE wait
                # doesn't include this memset (counter-based sync)
                ones_cols = v_sb.rearrange("p t (h c) -> p t h c", h=NHC)[
                    :, :, :, 64:65
                ]
                nc.vector.memset(ones_cols, 1.0)

                # Lean front: K c0 and Q c0 only; K c1-3, V, Q c1-3 are
                # background work drained into the first attention chunk
                # (anything emitted before the first scores would drag
                # their dependency counters up)
                # first chunks DMA'd in et-halves: the 8-MM accumulation
                # consumes et-tiles in order, so it can start on the first
                # half while the second is still in flight
                xs0 = xpool.tile([128, ET, 512], BF16, tag="x", name="xk0")
                nc.sync.dma_start(out=xs0[:, 0:4, :], in_=xk_d[:, 0, 0:4, :])
                nc.sync.dma_start(out=xs0[:, 4:8, :], in_=xk_d[:, 0, 4:8, :])
                nc.sync.dma_start(out=wq_sb, in_=wq_d[:])
                nc.sync.dma_start(out=bk_sb, in_=bk_d[:])
                xsq0 = xpool.tile([128, ET, 512], BF16, tag="x", name="xq0")
                nc.sync.dma_start(out=xsq0[:, 0:4, :], in_=xq_d[:, 0, 0:4, :])
                nc.sync.dma_start(out=xsq0[:, 4:8, :], in_=xq_d[:, 0, 4:8, :])
                nc.sync.dma_start(out=bq_sb, in_=bq_d[:])
                for f in range(2):
                    qk_mms(wk_sb, xs0, f, rings[f][:, 0, :])
                    nc.vector.tensor_scalar_add(
                        kt_c[0][:, f, :], rings[f][:, 0, :], bk_sb[:, f : f + 1]
                    )
                for f in range(2):
                    qk_mms(wq_sb, xsq0, f, rings[f][:, 1, :])
                    nc.vector.tensor_scalar_add(
                        qt_c[0][:, f, :], rings[f][:, 1, :], bq_sb[:, f : f + 1]
                    )

                bg = {
                    (0, 0): [[] for _ in range(8)],
                    (0, 2): [[] for _ in range(8)],
                }
                for c in (1, 2, 3):
                    bg[(0, 0)][c - 1] = proj_chunk_oprb_parts(
                        xk_d, wk_sb, bk_sb, kt_c[c], c, f"xk{c}"
                    )

                def wv_dma():
                    nc.sync.dma_start(out=wv_sb, in_=wv_d[:])
                    nc.sync.dma_start(out=bv_sb, in_=bv_d[:])
                    # replicate bv across partitions once: [128, FPC]
                    bp = oprb.tile([128, 512], F32, tag="oprb", name="bvrep_ps")
                    nc.tensor.matmul(
                        bp[:, 0:FPC], ones_sb[0:1, :], bv_sb, start=True, stop=True
                    )
                    nc.vector.tensor_copy(out=bvrep_sb, in_=bp[:, 0:FPC])

                def wo_dma():
                    nc.sync.dma_start(out=wo_sb, in_=wo_d[:])

                vslots = [
                    ((0, 0), 3), ((0, 0), 4), ((0, 0), 5), ((0, 0), 6),
                    ((0, 0), 7), ((0, 2), 0), ((0, 2), 1), ((0, 2), 2),
                ]
                bg[(0, 0)][3].append(wv_dma)
                for i, (key, g) in enumerate(vslots):
                    if i % 2 == 0:
                        bg[key][g].append(lambda tg=2 * i: v_dma(tg))
                    bg[key][g].append(lambda tt=2 * i: v_tt(tt))
                    bg[key][g].append(lambda tt=2 * i + 1: v_tt(tt))
                bg[(0, 2)][2].append(wo_dma)
                qparts = (
                    proj_chunk_oprb_parts(xq_d, wq_sb, bq_sb, qt_c[1], 1, "xq1")
                    + proj_chunk_oprb_parts(xq_d, wq_sb, bq_sb, qt_c[2], 2, "xq2")
                    + proj_chunk_oprb_parts(xq_d, wq_sb, bq_sb, qt_c[3], 3, "xq3")
                )
                for i, part in enumerate(qparts):
                    bg[(0, 2)][3 + min(i, 4)].append(part)

                pending = deque()
                seq = [0]

                def drain(n):
                    # pop in FIFO order, but never past an item whose
                    # producers (V token tiles) haven't been emitted yet --
                    # the dep tracker can only order reads against writes
                    # that already exist in the program
                    for _ in range(min(n, len(pending))):
                        ready, item = pending[0]
                        if not ready():
                            return
                        pending.popleft()
                        item()

                always = lambda: True  # noqa: E731

                def attnv_chunk(poh, h, pt, k0):
                    if "po" not in poh:
                        seq[0] += 1
                        poh["po"] = pops.tile(
                            [128, 512], F32, tag="po", name=f"po{seq[0]}"
                        )
                    po = poh["po"]
                    for kt in range(k0, k0 + 4):
                        nc.tensor.matmul(
                            po[0:65, :],
                            v_sb[:, kt, 65 * h : 65 * h + 65],
                            pt[:, kt, :],
                            start=(kt == 0),
                            stop=(kt == KT - 1),
                        )

                def norm_head(poh, h, at):
                    po = poh["po"]
                    seq[0] += 1
                    # row 64 of po = softmax denominator
                    rec = rpool.tile([128, 512], BF16, tag="rec", name=f"rec{seq[0]}")
                    with nc.allow_low_precision(reason="softmax denom"):
                        nc.vector.reciprocal(rec[64:65, :], po[64:65, :])
                    rb = oprb.tile([128, 512], F32, tag="oprb", name=f"rb{seq[0]}")
                    nc.tensor.matmul(
                        rb[0:64, :], ones_sb[64:65, 0:64], rec[64:65, :],
                        start=True, stop=True, tile_position=(64, 0),
                    )
                    rbs = rbpool.tile([64, 512], BF16, tag="rbs", name=f"rbs{seq[0]}")
                    nc.vector.tensor_copy(out=rbs, in_=rb[0:64, :])
                    r_sl = slice((h % 2) * 64, (h % 2) * 64 + 64)
                    nc.vector.tensor_mul(at[r_sl, h // 2, :], po[0:64, :], rbs)

                def outproj_half(at, qc, half):
                    ob = opool.tile(
                        [128, 4, 512], BF16, tag="ob", name=f"ob{qc}_{half}"
                    )
                    for fi, ft in enumerate(range(half * 4, half * 4 + 4)):
                        op = oprb.tile(
                            [128, 512], F32, tag="oprb", name=f"op{qc}_{ft}"
                        )
                        for ct in range(2):
                            nc.tensor.matmul(
                                op,
                                wo_sb[:, ct, ft, :],
                                at[:, ct, :],
                                start=(ct == 0),
                                stop=(ct == 1),
                            )
                        nc.vector.tensor_copy(out=ob[:, fi, :], in_=op)
                        if fi % 2 == 1:
                            nc.sync.dma_start(
                                out=out_d[
                                    :, half * 4 + fi - 1 : half * 4 + fi + 1, qc, :
                                ],
                                in_=ob[:, fi - 1 : fi + 1, :],
                            )

                def outproj_final(at, qc):
                    # last chunk: the score rings are free; use their 4 banks
                    # as rotating out-proj accumulators (no 2-buffer stall)
                    for half in range(2):
                        ob = opool.tile(
                            [128, 4, 512], BF16, tag="ob", name=f"obf{half}"
                        )
                        for fi, ft in enumerate(range(half * 4, half * 4 + 4)):
                            slot = rings[(ft % 4) // 2][:, ft % 2, :]
                            for ct in range(2):
                                nc.tensor.matmul(
                                    slot,
                                    wo_sb[:, ct, ft, :],
                                    at[:, ct, :],
                                    start=(ct == 0),
                                    stop=(ct == 1),
                                )
                            # tail is DVE-paced and ACT is idle by now:
                            # alternate copies across both engines, DMA out
                            # per f-tile so the drain starts immediately
                            if ft % 2 == 0:
                                nc.vector.tensor_copy(out=ob[:, fi, :], in_=slot)
                            else:
                                nc.scalar.copy(out=ob[:, fi, :], in_=slot)
                            nc.sync.dma_start(
                                out=out_d[:, half * 4 + fi : half * 4 + fi + 1, qc, :],
                                in_=ob[:, fi : fi + 1, :],
                            )

                for qc in range(QC):
                    at = apool.tile([128, 2, 512], BF16, tag="at")
                    for hp in (0, 2):
                        pts = [
                            ppool.tile([128, KT, 512], BF16, tag="p", name=f"p{qc}_{h}")
                            for h in (hp, hp + 1)
                        ]
                        bg_items = bg.get((qc, hp))
                        for g in range(KT // 2):
                            # scores for 2 kt x 2 heads; heads adjacent so the
                            # row-tiled pair can overlap in the PE on HW
                            for i in (0, 1):
                                kt = 2 * g + i
                                k_sl = slice((kt % 4) * 128, (kt % 4 + 1) * 128)
                                for j, h in enumerate((hp, hp + 1)):
                                    r_sl = slice((h % 2) * 64, (h % 2) * 64 + 64)
                                    nc.tensor.matmul(
                                        rings[j][:, i, :],
                                        kt_c[kt // 4][r_sl, h // 2, k_sl],
                                        qt_c[qc][r_sl, h // 2, :],
                                        start=True,
                                        stop=True,
                                        tile_position=((h % 2) * 64, 0),
                                    )
                            for j in (0, 1):
                                nc.scalar.activation(
                                    pts[j][:, 2 * g : 2 * g + 2, :],
                                    rings[j][:, :, :],
                                    EXP,
                                    scale=0.125,
                                )
                            if bg_items:
                                for item in bg_items[g]:
                                    item()
                            drain(2)
                        # defer attn@V + normalize: drained into the next
                        # pair's score/exp stream so the next scores stay
                        # inside the PE's limited lookahead window
                        for j, h in enumerate((hp, hp + 1)):
                            poh = {}
                            for k0 in range(0, KT, 4):
                                pending.append((
                                    lambda k0=k0: vtt_emitted[0] >= k0 + 4,
                                    lambda poh=poh, h=h, pt=pts[j], k0=k0:
                                        attnv_chunk(poh, h, pt, k0),
                                ))
                            pending.append((
                                always,
                                lambda poh=poh, h=h, at=at: norm_head(poh, h, at),
                            ))
                    if qc == QC - 1:
                        pending.append(
                            (always, lambda at=at, qc=qc: outproj_final(at, qc))
                        )
                    else:
                        pending.append(
                            (always, lambda at=at, qc=qc: outproj_half(at, qc, 0))
                        )
                        pending.append(
                            (always, lambda at=at, qc=qc: outproj_half(at, qc, 1))
                        )
                while pending:
                    ready, item = pending.popleft()
                    assert ready()
                    item()
    nc.compile()
    return nc


def _get_program():
    global _PROGRAM
    if _PROGRAM is None:
        _PROGRAM = _build_program()
    return _PROGRAM


def _prep_inputs(
    query, key, value, in_proj_weight, in_proj_bias, out_w,
    A_q, B_q, A_k, B_k, A_v, B_v,
):
    """Shard + lay out the full fp32 inputs into per-core input maps."""
    bf = ml_dtypes.bfloat16
    w_eff = {
        "q": in_proj_weight[0:E] + LORA_SCALE * (B_q @ A_q),
        "k": in_proj_weight[E : 2 * E] + LORA_SCALE * (B_k @ A_k),
        "v": in_proj_weight[2 * E :] + LORA_SCALE * (B_v @ A_v),
    }
    biases = {
        "q": in_proj_bias[0:E],
        "k": in_proj_bias[E : 2 * E],
        "v": in_proj_bias[2 * E :],
    }
    xin = {"q": query, "k": key, "v": value}
    # per-batch chunk-major layouts (shared by the 4 cores of each batch)
    xqk_b = {}
    xv_b = {}
    for b in range(B):
        for n in ("q", "k"):
            xb = np.ascontiguousarray(xin[n][:, b, :])  # [S, E]
            xqk_b[n, b] = np.ascontiguousarray(
                xb.reshape(QC, 512, ET, 128).transpose(3, 0, 2, 1)
            ).astype(bf)
        xb = np.ascontiguousarray(xin["v"][:, b, :])
        xv_b[b] = np.ascontiguousarray(
            xb.reshape(TT, 128, ET, 128).transpose(3, 2, 0, 1)
        ).astype(bf)

    in_maps = []
    for c in range(NCORES):
        b, g = c // 4, c % 4
        fsl = slice(g * FPC, (g + 1) * FPC)
        m = {
            "xq": xqk_b["q", b],
            "xk": xqk_b["k", b],
            "xv": xv_b[b],
        }
        for n in ("q", "k", "v"):
            wc = w_eff[n][fsl]  # [256, E]
            m["w" + n] = np.ascontiguousarray(
                wc.T.reshape(ET, 128, FPC).transpose(1, 0, 2)
            ).astype(bf)
        m["bq"] = np.ascontiguousarray(
            biases["q"][fsl].reshape(2, 128).T
        ).astype(np.float32)
        m["bk"] = np.ascontiguousarray(
            biases["k"][fsl].reshape(2, 128).T
        ).astype(np.float32)
        m["bv"] = np.ascontiguousarray(biases["v"][fsl].reshape(1, FPC)).astype(bf)
        wo_l = out_w[:, fsl]  # [E, 256]
        m["wo"] = np.ascontiguousarray(
            wo_l.reshape(OFT, 128, 2, 128).transpose(3, 2, 0, 1)
        ).astype(bf)
        in_maps.append(m)
    return in_maps


def kernel(
    query, key, value, in_proj_weight, in_proj_bias, out_w, out_b,
    A_q, B_q, A_k, B_k, A_v, B_v,
    _trace=False, _trace_kwargs=None,
):
    nc = _get_program()
    in_maps = _prep_inputs(
        query, key, value, in_proj_weight, in_proj_bias, out_w,
        A_q, B_q, A_k, B_k, A_v, B_v,
    )
    res = run_bass_kernel_spmd(
        nc, in_maps, list(range(NCORES)), trace=_trace, **(_trace_kwargs or {})
    )
    out = np.empty((S, B, E), np.float32)
    for b in range(B):
        acc = np.zeros((E, S), np.float32)
        for g in range(4):
            r = res.results[b * 4 + g]["out"]  # [128, OFT, QC, 512] bf16
            acc += np.asarray(r).astype(np.float32).transpose(1, 0, 2, 3).reshape(E, S)
        out[:, b, :] = acc.T + out_b[None, :]
    if _trace:
        return out, res
    return out



# revision 12
# speedup vs baseline: 1.1268x; 1.1268x over previous
"""LoRA MHA kernel for TRN2, batch x head-group parallel across 8 NeuronCores.

Problem: nn_LoRAMultiheadAttention (S=2048, B=2, E=1024, H=16, HD=64, rank=8).

Strategy (v3: residual-compensated fp8 DoubleRow + dual-engine softmax)
-----------------------------------------------------------------------
* Host folds the LoRA update into the frozen weights (W_eff = W + s*B@A) and
  rescales: W' = 32*W_eff, out_w' = out_w/32 (Q,K,V carry 32x; the exp scale
  absorbs the 1024x on scores).  K-bias is dropped (constant along keys ->
  cancels in softmax); V-bias is folded into out_b on the host
  (out_b' = out_b + out_w @ bv); only the Q bias stays on device.
* Hybrid sharding: core c handles batch c//4 and head group c%4 (4 heads).
* fp8e4 DoubleRow everywhere except the out-projection, with residual
  compensation so only the softmax P quantization carries real noise:
    - projections: x and W' are each stored as fp8 value+residual pairs
      (x8+dx, W8+dW); Q = W8x8 + W8dx + dWx8 in 12 DR matmuls per
      (chunk, head-group) vs 8 bf16 matmuls -- 25% fewer PE cycles and
      ~0.2% error (only the dW*dx cross term is dropped).
    - scores: per (head, key-tile) one DR matmul whose k-pair slots hold
      (K8, dK) against (Q8, Q8) -> K-exact, Q8-rounding only (~0.9%).
      The duplicate Q8 slot is produced by a tiny SBUF->SBUF DMA.
    - attn@V: V stored as value+residual (v8, dv), two DR matmuls per
      kt-pair accumulate V-exact attn (~0.2%); per head laid out as
      [64 V | ones | 15 zero-pad] = 80 cols (pad keeps the k-pair stride
      16B-aligned for the DoubleRow LDWEIGHTS ISA); the ones column makes
      attn@V also produce the softmax row-sum as output row 64.
* Softmax exp splits across two engines: ACT runs true exp -> fp8, DVE runs
  a one-instruction base-2 bit-trick (round(s*a+b) as int8 IS the fp8e4
  encoding of ~exp(s)).  P's fp8 rounding (~1.5%) is the dominant error
  source; total measured ~1.8e-2 against the 2e-2 gate.
* Normalization: reciprocal of row 64, PE-broadcast, multiply.
* Out-projection stays bf16; partials DMA'd out bf16 and summed on the host
  within each 4-core batch group (+ out_b').
* Emission is software-pipelined: attn@V / normalize / out-proj are deferred
  closures drained into the next head pair's score+exp stream; K c1-3 / V /
  Q c1-3 projections are background items inside the first attention chunk.
"""

import sys
from collections import deque

import numpy as np

if "/opt/trn_rl_repo" not in sys.path:
    sys.path.insert(0, "/opt/trn_rl_repo")

import ml_dtypes  # noqa: E402

import concourse.bass as bass  # noqa: E402
from concourse import bacc  # noqa: E402
import concourse.mybir as mybir  # noqa: E402
import concourse.tile as tile  # noqa: E402
from concourse.bass_utils import run_bass_kernel_spmd  # noqa: E402

S, B, E = 2048, 2, 1024
H, HD = 16, 64
RANK = 8
LORA_SCALE = 16.0 / RANK
NCORES = 8
NHC = 4            # heads per core
FPC = NHC * HD     # features per core = 256
ET = E // 128      # 8 contraction tiles for the projections
QC = S // 512      # 4 query/token chunks of 512
KT = S // 128      # 16 key tiles of 128
TT = S // 128      # 16 token tiles of 128 (V layout)
OFT = E // 128     # 8 output-feature tiles
VW = 80            # per-head V block: 64 V | 1 ones | 15 pad

WSCALE = 32.0
EXP_SCALE = 0.125 / (WSCALE * WSCALE)          # exp(s_raw * this)
DVE_A = EXP_SCALE * 1.4426950408889634 * 8.0   # fp8e4 bit-trick slope
DVE_B = 56.0 - 0.344                           # exponent bias + centering
DVE_FRAC = 0.46                                # fraction of exps on DVE

BF16 = mybir.dt.bfloat16
F32 = mybir.dt.float32
FP8 = mybir.dt.float8e4
I8 = mybir.dt.int8
EXP = mybir.ActivationFunctionType.Exp
DR = mybir.MatmulPerfMode.DoubleRow

_PROGRAM = None


def _build_program():
    nc = bacc.Bacc(None)

    xdr = {}
    for n in ("q", "k"):
        for p in ("8", "d"):
            xdr[n + p] = nc.dram_tensor(
                f"x{n}{p}", [128, QC, ET, 512], FP8, kind="ExternalInput"
            )
    xv8_d = nc.dram_tensor("xv8", [128, ET, S], FP8, kind="ExternalInput")
    xvd_d = nc.dram_tensor("xvd", [128, ET, S], FP8, kind="ExternalInput")
    wdr = {}
    for n in ("q", "k", "v"):
        for p in ("8", "d"):
            wdr[n + p] = nc.dram_tensor(
                f"w{n}{p}", [128, ET, FPC], FP8, kind="ExternalInput"
            )
    wo_d = nc.dram_tensor("wo", [128, 2, OFT, 128], BF16, kind="ExternalInput")
    bq_d = nc.dram_tensor("bq", [128, 2], F32, kind="ExternalInput")
    out_d = nc.dram_tensor("out", [128, OFT, QC, 512], BF16, kind="ExternalOutput")

    mult = mybir.AluOpType.mult
    addop = mybir.AluOpType.add

    with tile.TileContext(nc) as tc:
        with tc.tile_pool(name="singles", bufs=1) as singles:
            # Q^T/K^T per chunk: [p=(h%2)*64+d, head-group, slot, tok] where
            # kt slots = (K8, dK) and qt slots = (Q8, Q8-duplicate); the
            # score DoubleRow pair contracts K8*Q8 + dK*Q8 = K-exact scores.
            qt_c = [
                singles.tile([128, 2, 2, 512], FP8, tag=f"qt{c}", name=f"qt{c}")
                for c in range(QC)
            ]
            kt_c = [
                singles.tile([128, 2, 2, 512], FP8, tag=f"kt{c}", name=f"kt{c}")
                for c in range(QC)
            ]
            v8_sb = singles.tile([128, TT, VW * NHC], FP8, tag="v8")
            dv_sb = singles.tile([128, TT, VW * NHC], FP8, tag="dv")
            xv8_sb = singles.tile([128, ET, S], FP8, tag="xv8")
            xvd_sb = singles.tile([128, ET, S], FP8, tag="xvd")
            wsb = {}
            for n in ("q", "k", "v"):
                for p in ("8", "d"):
                    wsb[n + p] = singles.tile(
                        [128, ET, FPC], FP8, tag=f"w{n}{p}", name=f"w{n}{p}"
                    )
            wo_sb = singles.tile([128, 2, OFT, 128], BF16, tag="wo")
            bq_sb = singles.tile([128, 2], F32, tag="bq")
            ones_sb = singles.tile([128, 128], BF16, tag="ones")

            # DMA queue order = dependency order
            nc.sync.dma_start(out=wsb["k8"], in_=wdr["k8"][:])
            nc.sync.dma_start(out=wsb["kd"], in_=wdr["kd"][:])
            nc.vector.memset(ones_sb, 1.0)
            # V layout: zero everything (incl. pad + dv), then ones columns
            # in v8; gpsimd so neither DVE nor ACT pays for it
            nc.gpsimd.memset(v8_sb, 0.0)
            nc.gpsimd.memset(dv_sb, 0.0)
            v84 = v8_sb.rearrange("p t (h c) -> p t h c", h=NHC)
            dv4 = dv_sb.rearrange("p t (h c) -> p t h c", h=NHC)
            nc.gpsimd.memset(v84[:, :, :, 64:65], 1.0)

            with (
                tc.tile_pool(name="xchunks", bufs=10) as xpool,
                tc.tile_pool(name="ring_ps", bufs=2, space="PSUM") as ringp,
                tc.tile_pool(name="po_ps", bufs=2, space="PSUM") as pops,
                tc.tile_pool(name="oprb_ps", bufs=2, space="PSUM") as oprb,
                tc.tile_pool(name="p_sb", bufs=4) as ppool,
                tc.tile_pool(name="at_sb", bufs=2) as apool,
                tc.tile_pool(name="rec_sb", bufs=2) as rpool,
                tc.tile_pool(name="rbs_sb", bufs=3) as rbpool,
                tc.tile_pool(name="ob_sb", bufs=3) as opool,
            ):
                rings = [
                    ringp.tile([128, 2, 512], F32, tag="ring", name=f"ring{j}")
                    for j in range(2)
                ]

                exp_acc = [0.0]

                def emit_exp(dst, src):
                    # dst: pt slice [128,2,512] fp8; src: ring [128,2,512] PSUM
                    exp_acc[0] += DVE_FRAC
                    if exp_acc[0] >= 1.0:
                        exp_acc[0] -= 1.0
                        nc.vector.tensor_scalar(
                            dst.bitcast(I8), src, DVE_A, DVE_B, mult, addop
                        )
                    else:
                        nc.scalar.activation(dst, src, EXP, scale=EXP_SCALE)

                def qk_mms_comp(n, x8, xd, hg, ps):
                    # 12 DR matmuls: W8x8 + W8dx + dWx8 (pairs of e-tiles)
                    f_sl = slice(hg * 128, hg * 128 + 128)
                    terms = [
                        (wsb[n + "8"], x8), (wsb[n + "8"], xd),
                        (wsb[n + "d"], x8),
                    ]
                    nt = len(terms)
                    for ti, (w, x) in enumerate(terms):
                        for e2 in range(ET // 2):
                            nc.tensor.matmul(
                                ps,
                                w[:, 2 * e2 : 2 * e2 + 2, f_sl],
                                x[:, 2 * e2 : 2 * e2 + 2, :],
                                start=(ti == 0 and e2 == 0),
                                stop=(ti == nt - 1 and e2 == ET // 2 - 1),
                                perf_mode=DR,
                            )

                def proj_chunk_parts(n, dst, c, nm, is_q):
                    """Per-head-group closures for a projection chunk."""
                    xs = [None, None]

                    def part(hg):
                        if hg == 0:
                            xs[0] = xpool.tile(
                                [128, ET, 512], FP8, tag="x", name=f"{nm}8"
                            )
                            xs[1] = xpool.tile(
                                [128, ET, 512], FP8, tag="x", name=f"{nm}d"
                            )
                            for t, x_d in ((0, xdr[n + "8"]), (1, xdr[n + "d"])):
                                nc.sync.dma_start(
                                    out=xs[t][:, 0:4, :], in_=x_d[:, c, 0:4, :]
                                )
                                nc.sync.dma_start(
                                    out=xs[t][:, 4:8, :], in_=x_d[:, c, 4:8, :]
                                )
                        ps = oprb.tile([128, 512], F32, tag="oprb", name=f"{nm}_ps{hg}")
                        qk_mms_comp(n, xs[0], xs[1], hg, ps)
                        if is_q:
                            nc.vector.tensor_scalar_add(
                                dst[:, hg, 0, :], ps, bq_sb[:, hg : hg + 1]
                            )
                            nc.sync.dma_start(
                                out=dst[:, hg, 1, :], in_=dst[:, hg, 0, :]
                            )
                        else:
                            nc.scalar.copy(out=dst[:, hg, 0, :], in_=ps)
                            nc.vector.tensor_sub(
                                dst[:, hg, 1, :], ps, dst[:, hg, 0, :]
                            )

                    return [lambda hg=hg: part(hg) for hg in range(2)]

                def v_dma(tg):
                    for src, dst in ((xv8_d, xv8_sb), (xvd_d, xvd_sb)):
                        nc.sync.dma_start(
                            out=dst[:, :, tg * 512 : (tg + 1) * 512],
                            in_=src[:, :, tg * 512 : (tg + 1) * 512],
                        )

                vtt_emitted = [0]

                def v_tt_pair(tp):
                    vp = oprb.tile([128, 512], F32, tag="oprb", name=f"vp{tp}")
                    for half, tt in enumerate((2 * tp, 2 * tp + 1)):
                        t_sl = slice(tt * 128, (tt + 1) * 128)
                        terms = [
                            (xv8_sb, wsb["v8"]), (xvd_sb, wsb["v8"]),
                            (xv8_sb, wsb["vd"]),
                        ]
                        for ti, (x, w) in enumerate(terms):
                            for e2 in range(ET // 2):
                                nc.tensor.matmul(
                                    vp[:, half * 256 : half * 256 + 256],
                                    x[:, 2 * e2 : 2 * e2 + 2, t_sl],
                                    w[:, 2 * e2 : 2 * e2 + 2, :],
                                    start=(ti == 0 and e2 == 0),
                                    stop=(ti == 2 and e2 == ET // 2 - 1),
                                    perf_mode=DR,
                                )
                    src = vp.rearrange("p (t h c) -> p t h c", t=2, h=NHC)
                    d8 = v84[:, 2 * tp : 2 * tp + 2, :, 0:64]
                    dd = dv4[:, 2 * tp : 2 * tp + 2, :, 0:64]
                    nc.vector.tensor_copy(out=d8, in_=src)
                    nc.vector.tensor_sub(dd, src, d8)
                    vtt_emitted[0] += 2

                # warm the PE p-state during the initial input-DMA wait
                warm = oprb.tile([128, 512], F32, tag="oprb", name="warm")
                for w in range(48):
                    nc.tensor.matmul(
                        warm[:, 0:128], ones_sb, ones_sb,
                        start=(w == 0), stop=(w == 47),
                    )

                # Lean front: K c0 and Q c0 into the score rings' banks
                xk0 = [
                    xpool.tile([128, ET, 512], FP8, tag="x", name=f"xk0{p}")
                    for p in ("8", "d")
                ]
                for t, nmp in ((0, "k8"), (1, "kd")):
                    nc.sync.dma_start(out=xk0[t][:, 0:4, :], in_=xdr[nmp][:, 0, 0:4, :])
                    nc.sync.dma_start(out=xk0[t][:, 4:8, :], in_=xdr[nmp][:, 0, 4:8, :])
                nc.sync.dma_start(out=wsb["q8"], in_=wdr["q8"][:])
                nc.sync.dma_start(out=wsb["qd"], in_=wdr["qd"][:])
                xq0 = [
                    xpool.tile([128, ET, 512], FP8, tag="x", name=f"xq0{p}")
                    for p in ("8", "d")
                ]
                for t, nmp in ((0, "q8"), (1, "qd")):
                    nc.sync.dma_start(out=xq0[t][:, 0:4, :], in_=xdr[nmp][:, 0, 0:4, :])
                    nc.sync.dma_start(out=xq0[t][:, 4:8, :], in_=xdr[nmp][:, 0, 4:8, :])
                nc.sync.dma_start(out=bq_sb, in_=bq_d[:])
                for hg in range(2):
                    qk_mms_comp("k", xk0[0], xk0[1], hg, rings[0][:, hg, :])
                nc.scalar.copy(out=kt_c[0][:, :, 0, :], in_=rings[0])
                nc.vector.tensor_sub(
                    kt_c[0][:, :, 1, :], rings[0], kt_c[0][:, :, 0, :]
                )
                for hg in range(2):
                    qk_mms_comp("q", xq0[0], xq0[1], hg, rings[1][:, hg, :])
                    nc.vector.tensor_scalar_add(
                        qt_c[0][:, hg, 0, :], rings[1][:, hg, :],
                        bq_sb[:, hg : hg + 1],
                    )
                    nc.sync.dma_start(
                        out=qt_c[0][:, hg, 1, :], in_=qt_c[0][:, hg, 0, :]
                    )

                bg = {
                    (0, 0): [[] for _ in range(8)],
                    (0, 2): [[] for _ in range(8)],
                }
                for c in (1, 2, 3):
                    bg[(0, 0)][c - 1] = proj_chunk_parts(
                        "k", kt_c[c], c, f"xk{c}", False
                    )

                def wv_dma():
                    nc.sync.dma_start(out=wsb["v8"], in_=wdr["v8"][:])
                    nc.sync.dma_start(out=wsb["vd"], in_=wdr["vd"][:])

                def wo_dma():
                    nc.sync.dma_start(out=wo_sb, in_=wo_d[:])

                vslots = [
                    ((0, 0), 3), ((0, 0), 4), ((0, 0), 5), ((0, 0), 6),
                    ((0, 0), 7), ((0, 2), 0), ((0, 2), 1), ((0, 2), 2),
                ]
                bg[(0, 0)][3].append(wv_dma)
                for i, (key, g) in enumerate(vslots):
                    if i % 2 == 0:
                        bg[key][g].append(lambda tg=i // 2: v_dma(tg))
                    bg[key][g].append(lambda tp=i: v_tt_pair(tp))
                bg[(0, 2)][2].append(wo_dma)
                qparts = (
                    proj_chunk_parts("q", qt_c[1], 1, "xq1", True)
                    + proj_chunk_parts("q", qt_c[2], 2, "xq2", True)
                    + proj_chunk_parts("q", qt_c[3], 3, "xq3", True)
                )
                for i, part in enumerate(qparts):
                    bg[(0, 2)][3 + min(i, 4)].append(part)

                pending = deque()
                seq = [0]

                def drain(n):
                    for _ in range(min(n, len(pending))):
                        ready, item = pending[0]
                        if not ready():
                            return
                        pending.popleft()
                        item()

                always = lambda: True  # noqa: E731

                def attnv_chunk(poh, h, pt, c):
                    if "po" not in poh:
                        seq[0] += 1
                        poh["po"] = pops.tile(
                            [128, 512], F32, tag="po", name=f"po{seq[0]}"
                        )
                    po = poh["po"]
                    for m in (2 * c, 2 * c + 1):
                        for vi, vsrc in enumerate((v8_sb, dv_sb)):
                            nc.tensor.matmul(
                                po[0:VW, :],
                                vsrc[:, 2 * m : 2 * m + 2, VW * h : VW * h + VW],
                                pt[:, 2 * m : 2 * m + 2, :],
                                start=(m == 0 and vi == 0),
                                stop=(m == KT // 2 - 1 and vi == 1),
                                perf_mode=DR,
                            )

                def norm_head(poh, h, at):
                    po = poh["po"]
                    seq[0] += 1
                    rec = rpool.tile([128, 512], BF16, tag="rec", name=f"rec{seq[0]}")
                    with nc.allow_low_precision(reason="softmax denom"):
                        nc.vector.reciprocal(rec[64:65, :], po[64:65, :])
                    rb = oprb.tile([128, 512], F32, tag="oprb", name=f"rb{seq[0]}")
                    nc.tensor.matmul(
                        rb[0:64, :], ones_sb[64:65, 0:64], rec[64:65, :],
                        start=True, stop=True, tile_position=(64, 0),
                    )
                    rbs = rbpool.tile([64, 512], BF16, tag="rbs", name=f"rbs{seq[0]}")
                    nc.scalar.copy(out=rbs, in_=rb[0:64, :])
                    r_sl = slice((h % 2) * 64, (h % 2) * 64 + 64)
                    nc.vector.tensor_mul(at[r_sl, h // 2, :], po[0:64, :], rbs)

                def outproj_half(at, qc, half):
                    ob = opool.tile(
                        [128, 4, 512], BF16, tag="ob", name=f"ob{qc}_{half}"
                    )
                    for fi, ft in enumerate(range(half * 4, half * 4 + 4)):
                        op = oprb.tile(
                            [128, 512], F32, tag="oprb", name=f"op{qc}_{ft}"
                        )
                        for ct in range(2):
                            nc.tensor.matmul(
                                op,
                                wo_sb[:, ct, ft, :],
                                at[:, ct, :],
                                start=(ct == 0),
                                stop=(ct == 1),
                            )
                        nc.scalar.copy(out=ob[:, fi, :], in_=op)
                        if fi % 2 == 1:
                            nc.sync.dma_start(
                                out=out_d[
                                    :, half * 4 + fi - 1 : half * 4 + fi + 1, qc, :
                                ],
                                in_=ob[:, fi - 1 : fi + 1, :],
                            )

                def outproj_final(at, qc):
                    # last chunk: the score rings are free; use their 4 banks
                    # as rotating out-proj accumulators
                    for half in range(2):
                        ob = opool.tile(
                            [128, 4, 512], BF16, tag="ob", name=f"obf{half}"
                        )
                        for fi, ft in enumerate(range(half * 4, half * 4 + 4)):
                            slot = rings[(ft % 4) // 2][:, ft % 2, :]
                            for ct in range(2):
                                nc.tensor.matmul(
                                    slot,
                                    wo_sb[:, ct, ft, :],
                                    at[:, ct, :],
                                    start=(ct == 0),
                                    stop=(ct == 1),
                                )
                            if ft % 2 == 0:
                                nc.vector.tensor_copy(out=ob[:, fi, :], in_=slot)
                            else:
                                nc.scalar.copy(out=ob[:, fi, :], in_=slot)
                            nc.sync.dma_start(
                                out=out_d[:, half * 4 + fi : half * 4 + fi + 1, qc, :],
                                in_=ob[:, fi : fi + 1, :],
                            )

                for qc in range(QC):
                    at = apool.tile([128, 2, 512], BF16, tag="at")
                    for hp in (0, 2):
                        pts = [
                            ppool.tile([128, KT, 512], FP8, tag="p", name=f"p{qc}_{h}")
                            for h in (hp, hp + 1)
                        ]
                        bg_items = bg.get((qc, hp))
                        for g in range(KT // 2):
                            for j, h in enumerate((hp, hp + 1)):
                                r_sl = slice((h % 2) * 64, (h % 2) * 64 + 64)
                                hg = h // 2
                                for i in (0, 1):
                                    kt = 2 * g + i
                                    k_sl = slice((kt % 4) * 128, (kt % 4 + 1) * 128)
                                    nc.tensor.matmul(
                                        rings[j][:, i, :],
                                        kt_c[kt // 4][r_sl, hg, :, k_sl],
                                        qt_c[qc][r_sl, hg, :, :],
                                        start=True,
                                        stop=True,
                                        perf_mode=DR,
                                    )
                            for j in (0, 1):
                                emit_exp(pts[j][:, 2 * g : 2 * g + 2, :], rings[j])
                            if bg_items:
                                for item in bg_items[g]:
                                    item()
                            drain(2)
                        for j, h in enumerate((hp, hp + 1)):
                            poh = {}
                            for c in range(4):
                                pending.append((
                                    lambda c=c: vtt_emitted[0] >= 4 * c + 4,
                                    lambda poh=poh, h=h, pt=pts[j], c=c:
                                        attnv_chunk(poh, h, pt, c),
                                ))
                            pending.append((
                                always,
                                lambda poh=poh, h=h, at=at: norm_head(poh, h, at),
                            ))
                    if qc == QC - 1:
                        pending.append(
                            (always, lambda at=at, qc=qc: outproj_final(at, qc))
                        )
                    else:
                        pending.append(
                            (always, lambda at=at, qc=qc: outproj_half(at, qc, 0))
                        )
                        pending.append(
                            (always, lambda at=at, qc=qc: outproj_half(at, qc, 1))
                        )
                while pending:
                    ready, item = pending.popleft()
                    assert ready()
                    item()
    nc.compile()
    return nc


def _get_program():
    global _PROGRAM
    if _PROGRAM is None:
        _PROGRAM = _build_program()
    return _PROGRAM


def _f8_pair(a):
    f8 = ml_dtypes.float8_e4m3
    a8 = a.astype(f8)
    ad = (a - a8.astype(np.float32)).astype(f8)
    return a8, ad


def _prep_inputs(
    query, key, value, in_proj_weight, in_proj_bias, out_w,
    A_q, B_q, A_k, B_k, A_v, B_v,
):
    """Shard + lay out the full fp32 inputs into per-core input maps."""
    bf = ml_dtypes.bfloat16
    w_eff = {
        "q": (in_proj_weight[0:E] + LORA_SCALE * (B_q @ A_q)) * WSCALE,
        "k": (in_proj_weight[E : 2 * E] + LORA_SCALE * (B_k @ A_k)) * WSCALE,
        "v": (in_proj_weight[2 * E :] + LORA_SCALE * (B_v @ A_v)) * WSCALE,
    }
    bq_full = in_proj_bias[0:E] * WSCALE
    xin = {"q": query, "k": key, "v": value}
    # per-batch chunk-major layouts (shared by the 4 cores of each batch)
    xqk_b = {}
    xv_b = {}
    for b in range(B):
        for n in ("q", "k"):
            xb = np.ascontiguousarray(xin[n][:, b, :])  # [S, E]
            lay = np.ascontiguousarray(
                xb.reshape(QC, 512, ET, 128).transpose(3, 0, 2, 1)
            ).astype(np.float32)
            xqk_b[n, b] = _f8_pair(lay)
        xb = np.ascontiguousarray(xin["v"][:, b, :])
        lay = np.ascontiguousarray(
            xb.reshape(S, ET, 128).transpose(2, 1, 0)
        ).astype(np.float32)
        xv_b[b] = _f8_pair(lay)

    in_maps = []
    for c in range(NCORES):
        b, g = c // 4, c % 4
        fsl = slice(g * FPC, (g + 1) * FPC)
        m = {
            "xq8": xqk_b["q", b][0], "xqd": xqk_b["q", b][1],
            "xk8": xqk_b["k", b][0], "xkd": xqk_b["k", b][1],
            "xv8": xv_b[b][0], "xvd": xv_b[b][1],
        }
        for n in ("q", "k", "v"):
            wc = w_eff[n][fsl]  # [256, E]
            lay = np.ascontiguousarray(
                wc.T.reshape(ET, 128, FPC).transpose(1, 0, 2)
            ).astype(np.float32)
            m["w" + n + "8"], m["w" + n + "d"] = _f8_pair(lay)
        m["bq"] = np.ascontiguousarray(
            bq_full[fsl].reshape(2, 128).T
        ).astype(np.float32)
        wo_l = out_w[:, fsl] / WSCALE  # [E, 256]
        m["wo"] = np.ascontiguousarray(
            wo_l.reshape(OFT, 128, 2, 128).transpose(3, 2, 0, 1)
        ).astype(bf)
        in_maps.append(m)
    return in_maps


def kernel(
    query, key, value, in_proj_weight, in_proj_bias, out_w, out_b,
    A_q, B_q, A_k, B_k, A_v, B_v,
    _trace=False, _trace_kwargs=None,
):
    nc = _get_program()
    in_maps = _prep_inputs(
        query, key, value, in_proj_weight, in_proj_bias, out_w,
        A_q, B_q, A_k, B_k, A_v, B_v,
    )
    res = run_bass_kernel_spmd(
        nc, in_maps, list(range(NCORES)), trace=_trace, **(_trace_kwargs or {})
    )
    # host-side bias folding: V bias contributes out_w @ bv to every token
    bv = in_proj_bias[2 * E :]
    out_b_eff = out_b + out_w @ bv
    out = np.empty((S, B, E), np.float32)
    for b in range(B):
        acc = np.zeros((E, S), np.float32)
        for g in range(4):
            r = res.results[b * 4 + g]["out"]  # [128, OFT, QC, 512] bf16
            acc += np.asarray(r).astype(np.float32).transpose(1, 0, 2, 3).reshape(E, S)
        out[:, b, :] = acc.T + out_b_eff[None, :]
    if _trace:
        return out, res
    return out


# revision 17
# speedup vs baseline: 1.1602x; 1.0296x over previous
"""LoRA MHA kernel for TRN2, batch x head-group parallel across 8 NeuronCores.

Problem: nn_LoRAMultiheadAttention (S=2048, B=2, E=1024, H=16, HD=64, rank=8).

Strategy (v3: residual-compensated fp8 DoubleRow + dual-engine softmax)
-----------------------------------------------------------------------
* Host folds the LoRA update into the frozen weights (W_eff = W + s*B@A) and
  rescales: W' = 32*W_eff, out_w' = out_w/32 (Q,K,V carry 32x; the exp scale
  absorbs the 1024x on scores).  K-bias is dropped (constant along keys ->
  cancels in softmax); V-bias is folded into out_b on the host
  (out_b' = out_b + out_w @ bv); only the Q bias stays on device.
* Hybrid sharding: core c handles batch c//4 and head group c%4 (4 heads).
* fp8e4 DoubleRow everywhere except the out-projection, with residual
  compensation so only the softmax P quantization carries real noise:
    - projections: x and W' are each stored as fp8 value+residual pairs
      (x8+dx, W8+dW); Q = W8x8 + W8dx + dWx8 in 12 DR matmuls per
      (chunk, head-group) vs 8 bf16 matmuls -- 25% fewer PE cycles and
      ~0.2% error (only the dW*dx cross term is dropped).
    - scores: per (head, key-tile) one DR matmul whose k-pair slots hold
      (K8, dK) against (Q8, Q8) -> K-exact, Q8-rounding only (~0.9%).
      The duplicate Q8 slot is produced by a tiny SBUF->SBUF DMA.
    - attn@V: V stored as value+residual (v8, dv), two DR matmuls per
      kt-pair accumulate V-exact attn (~0.2%); per head laid out as
      [64 V | ones | 15 zero-pad] = 80 cols (pad keeps the k-pair stride
      16B-aligned for the DoubleRow LDWEIGHTS ISA); the ones column makes
      attn@V also produce the softmax row-sum as output row 64.
* Softmax exp splits across two engines: ACT runs true exp -> fp8, DVE runs
  a one-instruction base-2 bit-trick (round(s*a+b) as int8 IS the fp8e4
  encoding of ~exp(s)).  P's fp8 rounding (~1.5%) is the dominant error
  source; total measured ~1.8e-2 against the 2e-2 gate.
* Normalization: reciprocal of row 64, PE-broadcast, multiply.
* Out-projection stays bf16; partials DMA'd out bf16 and summed on the host
  within each 4-core batch group (+ out_b').
* Emission is software-pipelined: attn@V / normalize / out-proj are deferred
  closures drained into the next head pair's score+exp stream; K c1-3 / V /
  Q c1-3 projections are background items inside the first attention chunk.
"""

import sys
from collections import deque

import numpy as np

if "/opt/trn_rl_repo" not in sys.path:
    sys.path.insert(0, "/opt/trn_rl_repo")

import ml_dtypes  # noqa: E402

import concourse.bass as bass  # noqa: E402
from concourse import bacc  # noqa: E402
import concourse.mybir as mybir  # noqa: E402
import concourse.tile as tile  # noqa: E402
from concourse.bass_utils import run_bass_kernel_spmd  # noqa: E402

S, B, E = 2048, 2, 1024
H, HD = 16, 64
RANK = 8
LORA_SCALE = 16.0 / RANK
NCORES = 8
NHC = 4            # heads per core
FPC = NHC * HD     # features per core = 256
ET = E // 128      # 8 contraction tiles for the projections
QC = S // 512      # 4 query/token chunks of 512
KT = S // 128      # 16 key tiles of 128
TT = S // 128      # 16 token tiles of 128 (V layout)
OFT = E // 128     # 8 output-feature tiles
VW = 80            # per-head V block: 64 V | 1 ones | 15 pad

WSCALE = 32.0
EXP_SCALE = 0.125 / (WSCALE * WSCALE)          # exp(s_raw * this)
DVE_A = EXP_SCALE * 1.4426950408889634 * 8.0   # fp8e4 bit-trick slope
DVE_B = 56.0 - 0.344                           # exponent bias + centering
DVE_FRAC = 0.42                                # fraction of exps on DVE

BF16 = mybir.dt.bfloat16
F32 = mybir.dt.float32
FP8 = mybir.dt.float8e4
I8 = mybir.dt.int8
EXP = mybir.ActivationFunctionType.Exp
DR = mybir.MatmulPerfMode.DoubleRow

_PROGRAM = None


def _build_program():
    nc = bacc.Bacc(None)

    xdr = {}
    for n in ("q", "k"):
        for p in ("8", "d"):
            xdr[n + p] = nc.dram_tensor(
                f"x{n}{p}", [128, QC, ET, 512], FP8, kind="ExternalInput"
            )
    xv8_d = nc.dram_tensor("xv8", [128, ET, S], FP8, kind="ExternalInput")
    xvd_d = nc.dram_tensor("xvd", [128, ET, S], FP8, kind="ExternalInput")
    wdr = {}
    for n in ("q", "k", "v"):
        for p in ("8", "d"):
            wdr[n + p] = nc.dram_tensor(
                f"w{n}{p}", [128, ET, FPC], FP8, kind="ExternalInput"
            )
    wo_d = nc.dram_tensor("wo", [128, 2, OFT, 128], BF16, kind="ExternalInput")
    bq_d = nc.dram_tensor("bq", [128, 2], F32, kind="ExternalInput")
    out_d = nc.dram_tensor("out", [128, OFT, QC, 512], BF16, kind="ExternalOutput")

    mult = mybir.AluOpType.mult
    addop = mybir.AluOpType.add

    with tile.TileContext(nc) as tc:
        with tc.tile_pool(name="singles", bufs=1) as singles:
            # Q^T/K^T per chunk: [p=(h%2)*64+d, head-group, slot, tok] where
            # kt slots = (K8, dK) and qt slots = (Q8, Q8-duplicate); the
            # score DoubleRow pair contracts K8*Q8 + dK*Q8 = K-exact scores.
            qt_c = [
                singles.tile([128, 2, 2, 512], FP8, tag=f"qt{c}", name=f"qt{c}")
                for c in range(QC)
            ]
            kt_c = [
                singles.tile([128, 2, 2, 512], FP8, tag=f"kt{c}", name=f"kt{c}")
                for c in range(QC)
            ]
            v8_sb = singles.tile([128, TT, VW * NHC], FP8, tag="v8")
            dv_sb = singles.tile([128, TT, VW * NHC], FP8, tag="dv")
            xv8_sb = singles.tile([128, ET, S], FP8, tag="xv8")
            xvd_sb = singles.tile([128, ET, S], FP8, tag="xvd")
            wsb = {}
            for n in ("q", "k", "v"):
                for p in ("8", "d"):
                    wsb[n + p] = singles.tile(
                        [128, ET, FPC], FP8, tag=f"w{n}{p}", name=f"w{n}{p}"
                    )
            wo_sb = singles.tile([128, 2, OFT, 128], BF16, tag="wo")
            bq_sb = singles.tile([128, 2], F32, tag="bq")
            ones_sb = singles.tile([128, 128], BF16, tag="ones")

            # DMA queue order = dependency order
            nc.sync.dma_start(out=wsb["k8"], in_=wdr["k8"][:])
            nc.sync.dma_start(out=wsb["kd"], in_=wdr["kd"][:])
            nc.vector.memset(ones_sb, 1.0)
            # V layout: zero everything (incl. pad + dv), then ones columns
            # in v8; gpsimd so neither DVE nor ACT pays for it
            nc.gpsimd.memset(v8_sb, 0.0)
            nc.gpsimd.memset(dv_sb, 0.0)
            v84 = v8_sb.rearrange("p t (h c) -> p t h c", h=NHC)
            dv4 = dv_sb.rearrange("p t (h c) -> p t h c", h=NHC)
            nc.gpsimd.memset(v84[:, :, :, 64:65], 1.0)

            with (
                tc.tile_pool(name="xchunks", bufs=10) as xpool,
                tc.tile_pool(name="ring_ps", bufs=2, space="PSUM") as ringp,
                tc.tile_pool(name="po_ps", bufs=2, space="PSUM") as pops,
                tc.tile_pool(name="oprb_ps", bufs=2, space="PSUM") as oprb,
                tc.tile_pool(name="p_sb", bufs=4) as ppool,
                tc.tile_pool(name="at_sb", bufs=2) as apool,
                tc.tile_pool(name="rec_sb", bufs=2) as rpool,
                tc.tile_pool(name="rbs_sb", bufs=3) as rbpool,
                tc.tile_pool(name="ob_sb", bufs=3) as opool,
            ):
                rings = [
                    ringp.tile([128, 2, 512], F32, tag="ring", name=f"ring{j}")
                    for j in range(2)
                ]

                exp_acc = [0.0]

                def emit_exp(dst, src, j):
                    # dst: pt slice [128,2,512] fp8; src: ring [128,2,512] PSUM
                    exp_acc[0] += DVE_FRAC
                    if exp_acc[0] >= 1.0:
                        exp_acc[0] -= 1.0
                        nc.vector.tensor_scalar(
                            dst.bitcast(I8), src, DVE_A, DVE_B, mult, addop
                        )
                    else:
                        nc.scalar.activation(dst, src, EXP, scale=EXP_SCALE)

                def qk_mms_comp(n, x8, xd, hg, ps):
                    # 12 DR matmuls: W8x8 + W8dx + dWx8 (pairs of e-tiles)
                    f_sl = slice(hg * 128, hg * 128 + 128)
                    terms = [
                        (wsb[n + "8"], x8), (wsb[n + "8"], xd),
                        (wsb[n + "d"], x8),
                    ]
                    nt = len(terms)
                    for ti, (w, x) in enumerate(terms):
                        for e2 in range(ET // 2):
                            nc.tensor.matmul(
                                ps,
                                w[:, 2 * e2 : 2 * e2 + 2, f_sl],
                                x[:, 2 * e2 : 2 * e2 + 2, :],
                                start=(ti == 0 and e2 == 0),
                                stop=(ti == nt - 1 and e2 == ET // 2 - 1),
                                perf_mode=DR,
                            )

                def proj_chunk_parts(n, dst, c, nm, is_q):
                    """Per-head-group closures for a projection chunk."""
                    xs = [None, None]

                    def part(hg):
                        if hg == 0:
                            xs[0] = xpool.tile(
                                [128, ET, 512], FP8, tag="x", name=f"{nm}8"
                            )
                            xs[1] = xpool.tile(
                                [128, ET, 512], FP8, tag="x", name=f"{nm}d"
                            )
                            for t, x_d in ((0, xdr[n + "8"]), (1, xdr[n + "d"])):
                                nc.sync.dma_start(
                                    out=xs[t][:, 0:4, :], in_=x_d[:, c, 0:4, :]
                                )
                                nc.sync.dma_start(
                                    out=xs[t][:, 4:8, :], in_=x_d[:, c, 4:8, :]
                                )
                        ps = oprb.tile([128, 512], F32, tag="oprb", name=f"{nm}_ps{hg}")
                        qk_mms_comp(n, xs[0], xs[1], hg, ps)
                        if is_q:
                            nc.vector.tensor_scalar_add(
                                dst[:, hg, 0, :], ps, bq_sb[:, hg : hg + 1]
                            )
                            nc.sync.dma_start(
                                out=dst[:, hg, 1, :], in_=dst[:, hg, 0, :]
                            )
                        else:
                            nc.scalar.copy(out=dst[:, hg, 0, :], in_=ps)
                            nc.vector.tensor_sub(
                                dst[:, hg, 1, :], ps, dst[:, hg, 0, :]
                            )

                    return [lambda hg=hg: part(hg) for hg in range(2)]

                def v_dma(tg):
                    for src, dst in ((xv8_d, xv8_sb), (xvd_d, xvd_sb)):
                        nc.sync.dma_start(
                            out=dst[:, :, tg * 512 : (tg + 1) * 512],
                            in_=src[:, :, tg * 512 : (tg + 1) * 512],
                        )

                vtt_emitted = [0]

                def v_tt_pair(tp):
                    vp = oprb.tile([128, 512], F32, tag="oprb", name=f"vp{tp}")
                    for half, tt in enumerate((2 * tp, 2 * tp + 1)):
                        t_sl = slice(tt * 128, (tt + 1) * 128)
                        terms = [
                            (xv8_sb, wsb["v8"]), (xvd_sb, wsb["v8"]),
                            (xv8_sb, wsb["vd"]),
                        ]
                        for ti, (x, w) in enumerate(terms):
                            for e2 in range(ET // 2):
                                nc.tensor.matmul(
                                    vp[:, half * 256 : half * 256 + 256],
                                    x[:, 2 * e2 : 2 * e2 + 2, t_sl],
                                    w[:, 2 * e2 : 2 * e2 + 2, :],
                                    start=(ti == 0 and e2 == 0),
                                    stop=(ti == 2 and e2 == ET // 2 - 1),
                                    perf_mode=DR,
                                )
                    src = vp.rearrange("p (t h c) -> p t h c", t=2, h=NHC)
                    d8 = v84[:, 2 * tp : 2 * tp + 2, :, 0:64]
                    dd = dv4[:, 2 * tp : 2 * tp + 2, :, 0:64]
                    nc.scalar.copy(out=d8, in_=src)
                    nc.vector.tensor_sub(dd, src, d8)
                    vtt_emitted[0] += 2

                # warm the PE p-state during the initial input-DMA wait
                warm = oprb.tile([128, 512], F32, tag="oprb", name="warm")
                for w in range(48):
                    nc.tensor.matmul(
                        warm[:, 0:128], ones_sb, ones_sb,
                        start=(w == 0), stop=(w == 47),
                    )

                # Lean front: K c0 and Q c0 into the score rings' banks
                xk0 = [
                    xpool.tile([128, ET, 512], FP8, tag="x", name=f"xk0{p}")
                    for p in ("8", "d")
                ]
                for t, nmp in ((0, "k8"), (1, "kd")):
                    nc.sync.dma_start(out=xk0[t][:, 0:4, :], in_=xdr[nmp][:, 0, 0:4, :])
                    nc.sync.dma_start(out=xk0[t][:, 4:8, :], in_=xdr[nmp][:, 0, 4:8, :])
                nc.sync.dma_start(out=wsb["q8"], in_=wdr["q8"][:])
                nc.sync.dma_start(out=wsb["qd"], in_=wdr["qd"][:])
                xq0 = [
                    xpool.tile([128, ET, 512], FP8, tag="x", name=f"xq0{p}")
                    for p in ("8", "d")
                ]
                for t, nmp in ((0, "q8"), (1, "qd")):
                    nc.sync.dma_start(out=xq0[t][:, 0:4, :], in_=xdr[nmp][:, 0, 0:4, :])
                    nc.sync.dma_start(out=xq0[t][:, 4:8, :], in_=xdr[nmp][:, 0, 4:8, :])
                nc.sync.dma_start(out=bq_sb, in_=bq_d[:])
                for hg in range(2):
                    qk_mms_comp("k", xk0[0], xk0[1], hg, rings[0][:, hg, :])
                nc.scalar.copy(out=kt_c[0][:, :, 0, :], in_=rings[0])
                nc.vector.tensor_sub(
                    kt_c[0][:, :, 1, :], rings[0], kt_c[0][:, :, 0, :]
                )
                for hg in range(2):
                    qk_mms_comp("q", xq0[0], xq0[1], hg, rings[1][:, hg, :])
                    nc.vector.tensor_scalar_add(
                        qt_c[0][:, hg, 0, :], rings[1][:, hg, :],
                        bq_sb[:, hg : hg + 1],
                    )
                    nc.sync.dma_start(
                        out=qt_c[0][:, hg, 1, :], in_=qt_c[0][:, hg, 0, :]
                    )

                bg = {
                    (0, 0): [[] for _ in range(8)],
                    (0, 2): [[] for _ in range(8)],
                }
                for c in (1, 2, 3):
                    bg[(0, 0)][c - 1] = proj_chunk_parts(
                        "k", kt_c[c], c, f"xk{c}", False
                    )

                def wv_dma():
                    nc.sync.dma_start(out=wsb["v8"], in_=wdr["v8"][:])
                    nc.sync.dma_start(out=wsb["vd"], in_=wdr["vd"][:])

                def wo_dma():
                    nc.sync.dma_start(out=wo_sb, in_=wo_d[:])

                vslots = [
                    ((0, 0), 3), ((0, 0), 4), ((0, 0), 5), ((0, 0), 6),
                    ((0, 0), 7), ((0, 2), 0), ((0, 2), 1), ((0, 2), 2),
                ]
                bg[(0, 0)][3].append(wv_dma)
                for i, (key, g) in enumerate(vslots):
                    if i % 2 == 0:
                        bg[key][g].append(lambda tg=i // 2: v_dma(tg))
                    bg[key][g].append(lambda tp=i: v_tt_pair(tp))
                bg[(0, 2)][2].append(wo_dma)
                qparts = (
                    proj_chunk_parts("q", qt_c[1], 1, "xq1", True)
                    + proj_chunk_parts("q", qt_c[2], 2, "xq2", True)
                    + proj_chunk_parts("q", qt_c[3], 3, "xq3", True)
                )
                for i, part in enumerate(qparts):
                    bg[(0, 2)][3 + min(i, 4)].append(part)

                pending = deque()
                seq = [0]

                def drain(n):
                    for _ in range(min(n, len(pending))):
                        ready, item = pending[0]
                        if not ready():
                            return
                        pending.popleft()
                        item()

                always = lambda: True  # noqa: E731

                def attnv_chunk(poh, h, pt, c):
                    if "po" not in poh:
                        seq[0] += 1
                        poh["po"] = pops.tile(
                            [128, 512], F32, tag="po", name=f"po{seq[0]}"
                        )
                    po = poh["po"]
                    for m in (2 * c, 2 * c + 1):
                        for vi, vsrc in enumerate((v8_sb, dv_sb)):
                            nc.tensor.matmul(
                                po[0:VW, :],
                                vsrc[:, 2 * m : 2 * m + 2, VW * h : VW * h + VW],
                                pt[:, 2 * m : 2 * m + 2, :],
                                start=(m == 0 and vi == 0),
                                stop=(m == KT // 2 - 1 and vi == 1),
                                perf_mode=DR,
                            )

                def norm_head(poh, h, at):
                    po = poh["po"]
                    seq[0] += 1
                    rec = rpool.tile([128, 512], BF16, tag="rec", name=f"rec{seq[0]}")
                    with nc.allow_low_precision(reason="softmax denom"):
                        nc.vector.reciprocal(rec[64:65, :], po[64:65, :])
                    rb = oprb.tile([128, 512], F32, tag="oprb", name=f"rb{seq[0]}")
                    nc.tensor.matmul(
                        rb[0:64, :], ones_sb[64:65, 0:64], rec[64:65, :],
                        start=True, stop=True, tile_position=(64, 0),
                    )
                    rbs = rbpool.tile([64, 512], BF16, tag="rbs", name=f"rbs{seq[0]}")
                    nc.scalar.copy(out=rbs, in_=rb[0:64, :])
                    r_sl = slice((h % 2) * 64, (h % 2) * 64 + 64)
                    nc.vector.tensor_mul(at[r_sl, h // 2, :], po[0:64, :], rbs)

                def outproj_ft(obh, at, qc, half, fi):
                    if "ob" not in obh:
                        obh["ob"] = opool.tile(
                            [128, 4, 512], BF16, tag="ob", name=f"ob{qc}_{half}"
                        )
                    ob = obh["ob"]
                    ft = half * 4 + fi
                    op = oprb.tile(
                        [128, 512], F32, tag="oprb", name=f"op{qc}_{ft}"
                    )
                    for ct in range(2):
                        nc.tensor.matmul(
                            op,
                            wo_sb[:, ct, ft, :],
                            at[:, ct, :],
                            start=(ct == 0),
                            stop=(ct == 1),
                        )
                    nc.scalar.copy(out=ob[:, fi, :], in_=op)
                    if fi % 2 == 1:
                        nc.sync.dma_start(
                            out=out_d[:, ft - 1 : ft + 1, qc, :],
                            in_=ob[:, fi - 1 : fi + 1, :],
                        )

                def outproj_final(at, qc):
                    # last chunk: the score rings are free; use their 4 banks
                    # as rotating out-proj accumulators
                    for half in range(2):
                        ob = opool.tile(
                            [128, 4, 512], BF16, tag="ob", name=f"obf{half}"
                        )
                        for fi, ft in enumerate(range(half * 4, half * 4 + 4)):
                            slot = rings[(ft % 4) // 2][:, ft % 2, :]
                            for ct in range(2):
                                nc.tensor.matmul(
                                    slot,
                                    wo_sb[:, ct, ft, :],
                                    at[:, ct, :],
                                    start=(ct == 0),
                                    stop=(ct == 1),
                                )
                            if ft % 2 == 0:
                                nc.vector.tensor_copy(out=ob[:, fi, :], in_=slot)
                            else:
                                nc.scalar.copy(out=ob[:, fi, :], in_=slot)
                            nc.sync.dma_start(
                                out=out_d[:, half * 4 + fi : half * 4 + fi + 1, qc, :],
                                in_=ob[:, fi : fi + 1, :],
                            )

                for qc in range(QC):
                    at = apool.tile([128, 2, 512], BF16, tag="at")
                    for hp in (0, 2):
                        pts = [
                            ppool.tile([128, KT, 512], FP8, tag="p", name=f"p{qc}_{h}")
                            for h in (hp, hp + 1)
                        ]
                        bg_items = bg.get((qc, hp))
                        for g in range(KT // 2):
                            for j, h in enumerate((hp, hp + 1)):
                                r_sl = slice((h % 2) * 64, (h % 2) * 64 + 64)
                                hg = h // 2
                                for i in (0, 1):
                                    kt = 2 * g + i
                                    k_sl = slice((kt % 4) * 128, (kt % 4 + 1) * 128)
                                    nc.tensor.matmul(
                                        rings[j][:, i, :],
                                        kt_c[kt // 4][r_sl, hg, :, k_sl],
                                        qt_c[qc][r_sl, hg, :, :],
                                        start=True,
                                        stop=True,
                                        perf_mode=DR,
                                    )
                            for j in (0, 1):
                                emit_exp(pts[j][:, 2 * g : 2 * g + 2, :], rings[j], j)
                            if bg_items:
                                for item in bg_items[g]:
                                    item()
                            drain(2)
                        for j, h in enumerate((hp, hp + 1)):
                            poh = {}
                            for c in range(4):
                                pending.append((
                                    lambda c=c: vtt_emitted[0] >= 4 * c + 4,
                                    lambda poh=poh, h=h, pt=pts[j], c=c:
                                        attnv_chunk(poh, h, pt, c),
                                ))
                            pending.append((
                                always,
                                lambda poh=poh, h=h, at=at: norm_head(poh, h, at),
                            ))
                    if qc == QC - 1:
                        pending.append(
                            (always, lambda at=at, qc=qc: outproj_final(at, qc))
                        )
                    else:
                        for half in range(2):
                            obh = {}
                            for fi in range(4):
                                pending.append((
                                    always,
                                    lambda obh=obh, at=at, qc=qc, half=half, fi=fi:
                                        outproj_ft(obh, at, qc, half, fi),
                                ))
                while pending:
                    ready, item = pending.popleft()
                    assert ready()
                    item()
    nc.compile()
    return nc


def _get_program():
    global _PROGRAM
    if _PROGRAM is None:
        _PROGRAM = _build_program()
    return _PROGRAM


def _f8_pair(a):
    f8 = ml_dtypes.float8_e4m3
    a8 = a.astype(f8)
    ad = (a - a8.astype(np.float32)).astype(f8)
    return a8, ad


def _prep_inputs(
    query, key, value, in_proj_weight, in_proj_bias, out_w,
    A_q, B_q, A_k, B_k, A_v, B_v,
):
    """Shard + lay out the full fp32 inputs into per-core input maps."""
    bf = ml_dtypes.bfloat16
    w_eff = {
        "q": (in_proj_weight[0:E] + LORA_SCALE * (B_q @ A_q)) * WSCALE,
        "k": (in_proj_weight[E : 2 * E] + LORA_SCALE * (B_k @ A_k)) * WSCALE,
        "v": (in_proj_weight[2 * E :] + LORA_SCALE * (B_v @ A_v)) * WSCALE,
    }
    bq_full = in_proj_bias[0:E] * WSCALE
    xin = {"q": query, "k": key, "v": value}
    # per-batch chunk-major layouts (shared by the 4 cores of each batch)
    xqk_b = {}
    xv_b = {}
    for b in range(B):
        for n in ("q", "k"):
            xb = np.ascontiguousarray(xin[n][:, b, :])  # [S, E]
            lay = np.ascontiguousarray(
                xb.reshape(QC, 512, ET, 128).transpose(3, 0, 2, 1)
            ).astype(np.float32)
            xqk_b[n, b] = _f8_pair(lay)
        xb = np.ascontiguousarray(xin["v"][:, b, :])
        lay = np.ascontiguousarray(
            xb.reshape(S, ET, 128).transpose(2, 1, 0)
        ).astype(np.float32)
        xv_b[b] = _f8_pair(lay)

    in_maps = []
    for c in range(NCORES):
        b, g = c // 4, c % 4
        fsl = slice(g * FPC, (g + 1) * FPC)
        m = {
            "xq8": xqk_b["q", b][0], "xqd": xqk_b["q", b][1],
            "xk8": xqk_b["k", b][0], "xkd": xqk_b["k", b][1],
            "xv8": xv_b[b][0], "xvd": xv_b[b][1],
        }
        for n in ("q", "k", "v"):
            wc = w_eff[n][fsl]  # [256, E]
            lay = np.ascontiguousarray(
                wc.T.reshape(ET, 128, FPC).transpose(1, 0, 2)
            ).astype(np.float32)
            m["w" + n + "8"], m["w" + n + "d"] = _f8_pair(lay)
        m["bq"] = np.ascontiguousarray(
            bq_full[fsl].reshape(2, 128).T
        ).astype(np.float32)
        wo_l = out_w[:, fsl] / WSCALE  # [E, 256]
        m["wo"] = np.ascontiguousarray(
            wo_l.reshape(OFT, 128, 2, 128).transpose(3, 2, 0, 1)
        ).astype(bf)
        in_maps.append(m)
    return in_maps


def kernel(
    query, key, value, in_proj_weight, in_proj_bias, out_w, out_b,
    A_q, B_q, A_k, B_k, A_v, B_v,
    _trace=False, _trace_kwargs=None,
):
    nc = _get_program()
    in_maps = _prep_inputs(
        query, key, value, in_proj_weight, in_proj_bias, out_w,
        A_q, B_q, A_k, B_k, A_v, B_v,
    )
    res = run_bass_kernel_spmd(
        nc, in_maps, list(range(NCORES)), trace=_trace, **(_trace_kwargs or {})
    )
    # host-side bias folding: V bias contributes out_w @ bv to every token
    bv = in_proj_bias[2 * E :]
    out_b_eff = out_b + out_w @ bv
    out = np.empty((S, B, E), np.float32)
    for b in range(B):
        acc = np.zeros((E, S), np.float32)
        for g in range(4):
            r = res.results[b * 4 + g]["out"]  # [128, OFT, QC, 512] bf16
            acc += np.asarray(r).astype(np.float32).transpose(1, 0, 2, 3).reshape(E, S)
        out[:, b, :] = acc.T + out_b_eff[None, :]
    if _trace:
        return out, res
    return out


# revision 24
# speedup vs baseline: 1.2006x; 1.0348x over previous
"""LoRA MHA kernel for TRN2, batch x head-group parallel across 8 NeuronCores.

Problem: nn_LoRAMultiheadAttention (S=2048, B=2, E=1024, H=16, HD=64, rank=8).

Strategy (v3: residual-compensated fp8 DoubleRow + dual-engine softmax)
-----------------------------------------------------------------------
* Host folds the LoRA update into the frozen weights (W_eff = W + s*B@A) and
  rescales: W' = 32*W_eff, out_w' = out_w/32 (Q,K,V carry 32x; the exp scale
  absorbs the 1024x on scores).  K-bias is dropped (constant along keys ->
  cancels in softmax); V-bias is folded into out_b on the host
  (out_b' = out_b + out_w @ bv); only the Q bias stays on device.
* Hybrid sharding: core c handles batch c//4 and head group c%4 (4 heads).
* fp8e4 DoubleRow everywhere except the out-projection, with residual
  compensation so only the softmax P quantization carries real noise:
    - projections: x and W' are each stored as fp8 value+residual pairs
      (x8+dx, W8+dW); Q = W8x8 + W8dx + dWx8 in 12 DR matmuls per
      (chunk, head-group) vs 8 bf16 matmuls -- 25% fewer PE cycles and
      ~0.2% error (only the dW*dx cross term is dropped).
    - scores: per (head, key-tile) one DR matmul whose k-pair slots hold
      (K8, dK) against (Q8, Q8) -> K-exact, Q8-rounding only (~0.9%).
      The duplicate Q8 slot is produced by a tiny SBUF->SBUF DMA.
    - attn@V: V stored as value+residual (v8, dv), two DR matmuls per
      kt-pair accumulate V-exact attn (~0.2%); per head laid out as
      [64 V | ones | 15 zero-pad] = 80 cols (pad keeps the k-pair stride
      16B-aligned for the DoubleRow LDWEIGHTS ISA); the ones column makes
      attn@V also produce the softmax row-sum as output row 64.
* Softmax exp splits across two engines: ACT runs true exp -> fp8, DVE runs
  a one-instruction base-2 bit-trick (round(s*a+b) as int8 IS the fp8e4
  encoding of ~exp(s)).  P's fp8 rounding (~1.5%) is the dominant error
  source; total measured ~1.8e-2 against the 2e-2 gate.
* Normalization: reciprocal of row 64, PE-broadcast, multiply.
* Out-projection stays bf16; partials DMA'd out bf16 and summed on the host
  within each 4-core batch group (+ out_b').
* Emission is software-pipelined: attn@V / normalize / out-proj are deferred
  closures drained into the next head pair's score+exp stream; K c1-3 / V /
  Q c1-3 projections are background items inside the first attention chunk.
"""

import sys
from collections import deque

import numpy as np

if "/opt/trn_rl_repo" not in sys.path:
    sys.path.insert(0, "/opt/trn_rl_repo")

import ml_dtypes  # noqa: E402

import concourse.bass as bass  # noqa: E402
from concourse import bacc  # noqa: E402
import concourse.mybir as mybir  # noqa: E402
import concourse.tile as tile  # noqa: E402
from concourse.bass_utils import run_bass_kernel_spmd  # noqa: E402

S, B, E = 2048, 2, 1024
H, HD = 16, 64
RANK = 8
LORA_SCALE = 16.0 / RANK
NCORES = 8
NHC = 4            # heads per core
FPC = NHC * HD     # features per core = 256
ET = E // 128      # 8 contraction tiles for the projections
QC = S // 512      # 4 query/token chunks of 512
KT = S // 128      # 16 key tiles of 128
TT = S // 128      # 16 token tiles of 128 (V layout)
OFT = E // 128     # 8 output-feature tiles
VW = 80            # per-head V block: 64 V | 1 ones | 15 pad

WSCALE = 32.0
EXP_SCALE = 0.125 / (WSCALE * WSCALE)          # exp(s_raw * this)
DVE_A = EXP_SCALE * 1.4426950408889634 * 8.0   # fp8e4 bit-trick slope
DVE_B = 56.0 - 0.344                           # exponent bias + centering
DVE_FRAC = 0.44                                # fraction of exps on DVE

BF16 = mybir.dt.bfloat16
F32 = mybir.dt.float32
FP8 = mybir.dt.float8e4
I8 = mybir.dt.int8
EXP = mybir.ActivationFunctionType.Exp
DR = mybir.MatmulPerfMode.DoubleRow

_PROGRAM = None


def _build_program():
    nc = bacc.Bacc(None)

    xdr = {}
    for n in ("q", "k"):
        for p in ("8", "d"):
            xdr[n + p] = nc.dram_tensor(
                f"x{n}{p}", [128, QC, ET, 512], FP8, kind="ExternalInput"
            )
    xv8_d = nc.dram_tensor("xv8", [128, ET, S], FP8, kind="ExternalInput")
    xvd_d = nc.dram_tensor("xvd", [128, ET, S], FP8, kind="ExternalInput")
    wdr = {}
    for n in ("q", "k", "v"):
        for p in ("8", "d"):
            wdr[n + p] = nc.dram_tensor(
                f"w{n}{p}", [128, ET, FPC], FP8, kind="ExternalInput"
            )
    wo_d = nc.dram_tensor("wo", [128, 2, OFT, 128], BF16, kind="ExternalInput")
    bq_d = nc.dram_tensor("bq", [128, 2], F32, kind="ExternalInput")
    out_d = nc.dram_tensor("out", [128, OFT, QC, 512], BF16, kind="ExternalOutput")

    mult = mybir.AluOpType.mult
    addop = mybir.AluOpType.add

    with tile.TileContext(nc) as tc:
        with tc.tile_pool(name="singles", bufs=1) as singles:
            # Q^T/K^T per chunk: [p=(h%2)*64+d, head-group, slot, tok] where
            # kt slots = (K8, dK) and qt slots = (Q8, Q8-duplicate); the
            # score DoubleRow pair contracts K8*Q8 + dK*Q8 = K-exact scores.
            qt_c = [
                singles.tile([128, 2, 2, 512], FP8, tag=f"qt{c}", name=f"qt{c}")
                for c in range(QC)
            ]
            kt_c = [
                singles.tile([128, 2, 2, 512], FP8, tag=f"kt{c}", name=f"kt{c}")
                for c in range(QC)
            ]
            v8_sb = singles.tile([128, TT, VW * NHC], FP8, tag="v8")
            dv_sb = singles.tile([128, TT, VW * NHC], FP8, tag="dv")
            xv8_sb = singles.tile([128, ET, S], FP8, tag="xv8")
            xvd_sb = singles.tile([128, ET, S], FP8, tag="xvd")
            wsb = {}
            for n in ("q", "k", "v"):
                for p in ("8", "d"):
                    wsb[n + p] = singles.tile(
                        [128, ET, FPC], FP8, tag=f"w{n}{p}", name=f"w{n}{p}"
                    )
            wo_sb = singles.tile([128, 2, OFT, 128], BF16, tag="wo")
            bq_sb = singles.tile([128, 2], F32, tag="bq")
            ones_sb = singles.tile([128, 128], BF16, tag="ones")

            # DMA queue order = dependency order
            nc.sync.dma_start(out=wsb["k8"], in_=wdr["k8"][:])
            nc.sync.dma_start(out=wsb["kd"], in_=wdr["kd"][:])
            nc.vector.memset(ones_sb, 1.0)
            # V layout: zero everything (incl. pad + dv), then ones columns
            # in v8; gpsimd so neither DVE nor ACT pays for it
            nc.gpsimd.memset(v8_sb, 0.0)
            nc.gpsimd.memset(dv_sb, 0.0)
            v84 = v8_sb.rearrange("p t (h c) -> p t h c", h=NHC)
            dv4 = dv_sb.rearrange("p t (h c) -> p t h c", h=NHC)
            nc.gpsimd.memset(v84[:, :, :, 64:65], 1.0)

            with (
                tc.tile_pool(name="xchunks", bufs=10) as xpool,
                tc.tile_pool(name="ring_ps", bufs=2, space="PSUM") as ringp,
                tc.tile_pool(name="po_ps", bufs=2, space="PSUM") as pops,
                tc.tile_pool(name="oprb_ps", bufs=2, space="PSUM") as oprb,
                tc.tile_pool(name="p_sb", bufs=4) as ppool,
                tc.tile_pool(name="at_sb", bufs=2) as apool,
                tc.tile_pool(name="rec_sb", bufs=2) as rpool,
                tc.tile_pool(name="rbs_sb", bufs=3) as rbpool,
                tc.tile_pool(name="ob_sb", bufs=3) as opool,
            ):
                rings = [
                    ringp.tile([128, 2, 512], F32, tag="ring", name=f"ring{j}")
                    for j in range(2)
                ]

                exp_acc = [0.0]

                def emit_exp(dst, src, j):
                    # dst: pt slice [128,2,512] fp8; src: ring [128,2,512] PSUM
                    exp_acc[0] += DVE_FRAC
                    if exp_acc[0] >= 1.0:
                        exp_acc[0] -= 1.0
                        nc.vector.tensor_scalar(
                            dst.bitcast(I8), src, DVE_A, DVE_B, mult, addop
                        )
                    else:
                        nc.scalar.activation(dst, src, EXP, scale=EXP_SCALE)

                def qk_mms_comp(n, x8, xd, hg, ps):
                    # 12 DR matmuls: W8x8 + W8dx + dWx8 (pairs of e-tiles)
                    f_sl = slice(hg * 128, hg * 128 + 128)
                    terms = [
                        (wsb[n + "8"], x8), (wsb[n + "8"], xd),
                        (wsb[n + "d"], x8),
                    ]
                    nt = len(terms)
                    for ti, (w, x) in enumerate(terms):
                        for e2 in range(ET // 2):
                            nc.tensor.matmul(
                                ps,
                                w[:, 2 * e2 : 2 * e2 + 2, f_sl],
                                x[:, 2 * e2 : 2 * e2 + 2, :],
                                start=(ti == 0 and e2 == 0),
                                stop=(ti == nt - 1 and e2 == ET // 2 - 1),
                                perf_mode=DR,
                            )

                def proj_chunk_parts(n, dst, c, nm, is_q):
                    """Per-head-group closures for a projection chunk."""
                    xs = [None, None]

                    def part(hg):
                        if hg == 0:
                            xs[0] = xpool.tile(
                                [128, ET, 512], FP8, tag="x", name=f"{nm}8"
                            )
                            xs[1] = xpool.tile(
                                [128, ET, 512], FP8, tag="x", name=f"{nm}d"
                            )
                            for t, x_d in ((0, xdr[n + "8"]), (1, xdr[n + "d"])):
                                nc.sync.dma_start(
                                    out=xs[t][:, 0:4, :], in_=x_d[:, c, 0:4, :]
                                )
                                nc.sync.dma_start(
                                    out=xs[t][:, 4:8, :], in_=x_d[:, c, 4:8, :]
                                )
                        ps = oprb.tile([128, 512], F32, tag="oprb", name=f"{nm}_ps{hg}")
                        qk_mms_comp(n, xs[0], xs[1], hg, ps)
                        if is_q:
                            nc.vector.tensor_scalar_add(
                                dst[:, hg, 0, :], ps, bq_sb[:, hg : hg + 1]
                            )
                            nc.sync.dma_start(
                                out=dst[:, hg, 1, :], in_=dst[:, hg, 0, :]
                            )
                        else:
                            nc.scalar.copy(out=dst[:, hg, 0, :], in_=ps)
                            nc.vector.tensor_sub(
                                dst[:, hg, 1, :], ps, dst[:, hg, 0, :]
                            )

                    return [lambda hg=hg: part(hg) for hg in range(2)]

                def v_dma(tg):
                    for src, dst in ((xv8_d, xv8_sb), (xvd_d, xvd_sb)):
                        nc.sync.dma_start(
                            out=dst[:, :, tg * 512 : (tg + 1) * 512],
                            in_=src[:, :, tg * 512 : (tg + 1) * 512],
                        )

                vtt_emitted = [0]

                def v_tt_pair(tp):
                    vp = oprb.tile([128, 512], F32, tag="oprb", name=f"vp{tp}")
                    for half, tt in enumerate((2 * tp, 2 * tp + 1)):
                        t_sl = slice(tt * 128, (tt + 1) * 128)
                        terms = [
                            (xv8_sb, wsb["v8"]), (xvd_sb, wsb["v8"]),
                            (xv8_sb, wsb["vd"]),
                        ]
                        for ti, (x, w) in enumerate(terms):
                            for e2 in range(ET // 2):
                                nc.tensor.matmul(
                                    vp[:, half * 256 : half * 256 + 256],
                                    x[:, 2 * e2 : 2 * e2 + 2, t_sl],
                                    w[:, 2 * e2 : 2 * e2 + 2, :],
                                    start=(ti == 0 and e2 == 0),
                                    stop=(ti == 2 and e2 == ET // 2 - 1),
                                    perf_mode=DR,
                                )
                    src = vp.rearrange("p (t h c) -> p t h c", t=2, h=NHC)
                    d8 = v84[:, 2 * tp : 2 * tp + 2, :, 0:64]
                    dd = dv4[:, 2 * tp : 2 * tp + 2, :, 0:64]
                    nc.scalar.copy(out=d8, in_=src)
                    nc.vector.tensor_sub(dd, src, d8)
                    vtt_emitted[0] += 2

                # warm the PE p-state during the initial input-DMA wait
                warm = oprb.tile([128, 512], F32, tag="oprb", name="warm")
                for w in range(48):
                    nc.tensor.matmul(
                        warm[:, 0:128], ones_sb, ones_sb,
                        start=(w == 0), stop=(w == 47),
                    )

                # Lean front: K c0 and Q c0 into the score rings' banks
                xk0 = [
                    xpool.tile([128, ET, 512], FP8, tag="x", name=f"xk0{p}")
                    for p in ("8", "d")
                ]
                for t, nmp in ((0, "k8"), (1, "kd")):
                    nc.sync.dma_start(out=xk0[t][:, 0:4, :], in_=xdr[nmp][:, 0, 0:4, :])
                    nc.sync.dma_start(out=xk0[t][:, 4:8, :], in_=xdr[nmp][:, 0, 4:8, :])
                nc.sync.dma_start(out=wsb["q8"], in_=wdr["q8"][:])
                nc.sync.dma_start(out=wsb["qd"], in_=wdr["qd"][:])
                xq0 = [
                    xpool.tile([128, ET, 512], FP8, tag="x", name=f"xq0{p}")
                    for p in ("8", "d")
                ]
                for t, nmp in ((0, "q8"), (1, "qd")):
                    nc.sync.dma_start(out=xq0[t][:, 0:4, :], in_=xdr[nmp][:, 0, 0:4, :])
                    nc.sync.dma_start(out=xq0[t][:, 4:8, :], in_=xdr[nmp][:, 0, 4:8, :])
                nc.sync.dma_start(out=bq_sb, in_=bq_d[:])
                for hg in range(2):
                    qk_mms_comp("k", xk0[0], xk0[1], hg, rings[0][:, hg, :])
                nc.scalar.copy(out=kt_c[0][:, :, 0, :], in_=rings[0])
                nc.vector.tensor_sub(
                    kt_c[0][:, :, 1, :], rings[0], kt_c[0][:, :, 0, :]
                )
                for hg in range(2):
                    qk_mms_comp("q", xq0[0], xq0[1], hg, rings[1][:, hg, :])
                    nc.vector.tensor_scalar_add(
                        qt_c[0][:, hg, 0, :], rings[1][:, hg, :],
                        bq_sb[:, hg : hg + 1],
                    )
                    nc.sync.dma_start(
                        out=qt_c[0][:, hg, 1, :], in_=qt_c[0][:, hg, 0, :]
                    )

                bg = {
                    (0, 0): [[] for _ in range(8)],
                    (0, 2): [[] for _ in range(8)],
                    (1, 0): [[] for _ in range(8)],
                }
                for c in (1, 2, 3):
                    bg[(0, 0)][c - 1] = proj_chunk_parts(
                        "k", kt_c[c], c, f"xk{c}", False
                    )

                def wv_dma():
                    nc.sync.dma_start(out=wsb["v8"], in_=wdr["v8"][:])
                    nc.sync.dma_start(out=wsb["vd"], in_=wdr["vd"][:])

                def wo_dma():
                    nc.sync.dma_start(out=wo_sb, in_=wo_d[:])

                vslots = [
                    ((0, 0), 3), ((0, 0), 4), ((0, 0), 5), ((0, 0), 6),
                    ((0, 0), 7), ((0, 2), 0), ((1, 0), 0), ((1, 0), 2),
                ]
                bg[(0, 0)][3].append(wv_dma)
                for i, (key, g) in enumerate(vslots):
                    if i % 2 == 0:
                        bg[key][g].append(lambda tg=i // 2: v_dma(tg))
                    bg[key][g].append(lambda tp=i: v_tt_pair(tp))
                bg[(0, 2)][2].append(wo_dma)
                qparts = (
                    proj_chunk_parts("q", qt_c[1], 1, "xq1", True)
                    + proj_chunk_parts("q", qt_c[2], 2, "xq2", True)
                    + proj_chunk_parts("q", qt_c[3], 3, "xq3", True)
                )
                for i, part in enumerate(qparts[:3]):
                    bg[(0, 2)][3 + i].append(part)
                for i, part in enumerate(qparts[3:]):
                    bg[(1, 0)][1 + 2 * i].append(part)

                pending = deque()
                seq = [0]

                def drain(n):
                    for _ in range(min(n, len(pending))):
                        ready, item = pending[0]
                        if not ready():
                            return
                        pending.popleft()
                        item()

                always = lambda: True  # noqa: E731

                def attnv_chunk(poh, h, pt, c):
                    if "po" not in poh:
                        seq[0] += 1
                        poh["po"] = pops.tile(
                            [128, 512], F32, tag="po", name=f"po{seq[0]}"
                        )
                    po = poh["po"]
                    for m in (2 * c, 2 * c + 1):
                        for vi, vsrc in enumerate((v8_sb, dv_sb)):
                            nc.tensor.matmul(
                                po[0:VW, :],
                                vsrc[:, 2 * m : 2 * m + 2, VW * h : VW * h + VW],
                                pt[:, 2 * m : 2 * m + 2, :],
                                start=(m == 0 and vi == 0),
                                stop=(m == KT // 2 - 1 and vi == 1),
                                perf_mode=DR,
                            )

                def norm_head(poh, h, at):
                    po = poh["po"]
                    seq[0] += 1
                    rec = rpool.tile([128, 512], BF16, tag="rec", name=f"rec{seq[0]}")
                    with nc.allow_low_precision(reason="softmax denom"):
                        nc.vector.reciprocal(rec[64:65, :], po[64:65, :])
                    rb = oprb.tile([128, 512], F32, tag="oprb", name=f"rb{seq[0]}")
                    nc.tensor.matmul(
                        rb[0:64, :], ones_sb[64:65, 0:64], rec[64:65, :],
                        start=True, stop=True, tile_position=(64, 0),
                    )
                    rbs = rbpool.tile([64, 512], BF16, tag="rbs", name=f"rbs{seq[0]}")
                    nc.scalar.copy(out=rbs, in_=rb[0:64, :])
                    r_sl = slice((h % 2) * 64, (h % 2) * 64 + 64)
                    nc.vector.tensor_mul(at[r_sl, h // 2, :], po[0:64, :], rbs)

                def outproj_ft(obh, at, qc, half, fi):
                    if "ob" not in obh:
                        obh["ob"] = opool.tile(
                            [128, 4, 512], BF16, tag="ob", name=f"ob{qc}_{half}"
                        )
                    ob = obh["ob"]
                    ft = half * 4 + fi
                    op = oprb.tile(
                        [128, 512], F32, tag="oprb", name=f"op{qc}_{ft}"
                    )
                    for ct in range(2):
                        nc.tensor.matmul(
                            op,
                            wo_sb[:, ct, ft, :],
                            at[:, ct, :],
                            start=(ct == 0),
                            stop=(ct == 1),
                        )
                    nc.scalar.copy(out=ob[:, fi, :], in_=op)
                    if fi % 2 == 1:
                        nc.sync.dma_start(
                            out=out_d[:, ft - 1 : ft + 1, qc, :],
                            in_=ob[:, fi - 1 : fi + 1, :],
                        )

                def outproj_final(at, qc):
                    # last chunk: the score rings are free; use their 4 banks
                    # as rotating out-proj accumulators
                    for half in range(2):
                        ob = opool.tile(
                            [128, 4, 512], BF16, tag="ob", name=f"obf{half}"
                        )
                        for fi, ft in enumerate(range(half * 4, half * 4 + 4)):
                            slot = rings[(ft % 4) // 2][:, ft % 2, :]
                            for ct in range(2):
                                nc.tensor.matmul(
                                    slot,
                                    wo_sb[:, ct, ft, :],
                                    at[:, ct, :],
                                    start=(ct == 0),
                                    stop=(ct == 1),
                                )
                            if ft % 2 == 0:
                                nc.vector.tensor_copy(out=ob[:, fi, :], in_=slot)
                            else:
                                nc.scalar.copy(out=ob[:, fi, :], in_=slot)
                            nc.sync.dma_start(
                                out=out_d[:, half * 4 + fi : half * 4 + fi + 1, qc, :],
                                in_=ob[:, fi : fi + 1, :],
                            )

                for qc in range(QC):
                    at = apool.tile([128, 2, 512], BF16, tag="at")
                    for hp in (0, 2):
                        pts = [
                            ppool.tile([128, KT, 512], FP8, tag="p", name=f"p{qc}_{h}")
                            for h in (hp, hp + 1)
                        ]
                        bg_items = bg.get((qc, hp))
                        for g in range(KT // 2):
                            for j, h in enumerate((hp, hp + 1)):
                                r_sl = slice((h % 2) * 64, (h % 2) * 64 + 64)
                                hg = h // 2
                                for i in (0, 1):
                                    kt = 2 * g + i
                                    k_sl = slice((kt % 4) * 128, (kt % 4 + 1) * 128)
                                    nc.tensor.matmul(
                                        rings[j][:, i, :],
                                        kt_c[kt // 4][r_sl, hg, :, k_sl],
                                        qt_c[qc][r_sl, hg, :, :],
                                        start=True,
                                        stop=True,
                                        perf_mode=DR,
                                    )
                            for j in (0, 1):
                                emit_exp(pts[j][:, 2 * g : 2 * g + 2, :], rings[j], j)
                            if bg_items:
                                for item in bg_items[g]:
                                    item()
                            drain(2)
                        for j, h in enumerate((hp, hp + 1)):
                            poh = {}
                            for c in range(4):
                                pending.append((
                                    lambda c=c: vtt_emitted[0] >= 4 * c + 4,
                                    lambda poh=poh, h=h, pt=pts[j], c=c:
                                        attnv_chunk(poh, h, pt, c),
                                ))
                            pending.append((
                                always,
                                lambda poh=poh, h=h, at=at: norm_head(poh, h, at),
                            ))
                    if qc == QC - 1:
                        pending.append(
                            (always, lambda at=at, qc=qc: outproj_final(at, qc))
                        )
                    else:
                        for half in range(2):
                            obh = {}
                            for fi in range(4):
                                pending.append((
                                    always,
                                    lambda obh=obh, at=at, qc=qc, half=half, fi=fi:
                                        outproj_ft(obh, at, qc, half, fi),
                                ))
                while pending:
                    ready, item = pending.popleft()
                    assert ready()
                    item()
    nc.compile()
    return nc


def _get_program():
    global _PROGRAM
    if _PROGRAM is None:
        _PROGRAM = _build_program()
    return _PROGRAM


def _f8_pair(a):
    f8 = ml_dtypes.float8_e4m3
    a8 = a.astype(f8)
    ad = (a - a8.astype(np.float32)).astype(f8)
    return a8, ad


def _prep_inputs(
    query, key, value, in_proj_weight, in_proj_bias, out_w,
    A_q, B_q, A_k, B_k, A_v, B_v,
):
    """Shard + lay out the full fp32 inputs into per-core input maps."""
    bf = ml_dtypes.bfloat16
    w_eff = {
        "q": (in_proj_weight[0:E] + LORA_SCALE * (B_q @ A_q)) * WSCALE,
        "k": (in_proj_weight[E : 2 * E] + LORA_SCALE * (B_k @ A_k)) * WSCALE,
        "v": (in_proj_weight[2 * E :] + LORA_SCALE * (B_v @ A_v)) * WSCALE,
    }
    bq_full = in_proj_bias[0:E] * WSCALE
    xin = {"q": query, "k": key, "v": value}
    # per-batch chunk-major layouts (shared by the 4 cores of each batch)
    xqk_b = {}
    xv_b = {}
    for b in range(B):
        for n in ("q", "k"):
            xb = np.ascontiguousarray(xin[n][:, b, :])  # [S, E]
            lay = np.ascontiguousarray(
                xb.reshape(QC, 512, ET, 128).transpose(3, 0, 2, 1)
            ).astype(np.float32)
            xqk_b[n, b] = _f8_pair(lay)
        xb = np.ascontiguousarray(xin["v"][:, b, :])
        lay = np.ascontiguousarray(
            xb.reshape(S, ET, 128).transpose(2, 1, 0)
        ).astype(np.float32)
        xv_b[b] = _f8_pair(lay)

    in_maps = []
    for c in range(NCORES):
        b, g = c // 4, c % 4
        fsl = slice(g * FPC, (g + 1) * FPC)
        m = {
            "xq8": xqk_b["q", b][0], "xqd": xqk_b["q", b][1],
            "xk8": xqk_b["k", b][0], "xkd": xqk_b["k", b][1],
            "xv8": xv_b[b][0], "xvd": xv_b[b][1],
        }
        for n in ("q", "k", "v"):
            wc = w_eff[n][fsl]  # [256, E]
            lay = np.ascontiguousarray(
                wc.T.reshape(ET, 128, FPC).transpose(1, 0, 2)
            ).astype(np.float32)
            m["w" + n + "8"], m["w" + n + "d"] = _f8_pair(lay)
        m["bq"] = np.ascontiguousarray(
            bq_full[fsl].reshape(2, 128).T
        ).astype(np.float32)
        wo_l = out_w[:, fsl] / WSCALE  # [E, 256]
        m["wo"] = np.ascontiguousarray(
            wo_l.reshape(OFT, 128, 2, 128).transpose(3, 2, 0, 1)
        ).astype(bf)
        in_maps.append(m)
    return in_maps


def kernel(
    query, key, value, in_proj_weight, in_proj_bias, out_w, out_b,
    A_q, B_q, A_k, B_k, A_v, B_v,
    _trace=False, _trace_kwargs=None,
):
    nc = _get_program()
    in_maps = _prep_inputs(
        query, key, value, in_proj_weight, in_proj_bias, out_w,
        A_q, B_q, A_k, B_k, A_v, B_v,
    )
    res = run_bass_kernel_spmd(
        nc, in_maps, list(range(NCORES)), trace=_trace, **(_trace_kwargs or {})
    )
    # host-side bias folding: V bias contributes out_w @ bv to every token
    bv = in_proj_bias[2 * E :]
    out_b_eff = out_b + out_w @ bv
    out = np.empty((S, B, E), np.float32)
    for b in range(B):
        acc = np.zeros((E, S), np.float32)
        for g in range(4):
            r = res.results[b * 4 + g]["out"]  # [128, OFT, QC, 512] bf16
            acc += np.asarray(r).astype(np.float32).transpose(1, 0, 2, 3).reshape(E, S)
        out[:, b, :] = acc.T + out_b_eff[None, :]
    if _trace:
        return out, res
    return out
